# revision 4
# baseline (speedup 1.0000x reference)
"""Trainium2 Bass kernel for nn_CSRA (SS2D/Mamba-style cross-scan module).

Self-contained: builds an SPMD Bass/Tile program for 8 NeuronCores, shards
inputs host-side, runs via run_bass_kernel_spmd, reassembles the output.

Sharding: core c -> (b = c//4, dh = (c%4)//2, nh = c%2).
Every core: full pre-stage for its batch b (BN+pre_proj+in_proj+depthwise
conv, all-DI); then 4 scan-direction sections over its (d-half, n-half)
lanes; per-chunk n-reduction via identity-matmul PSUM accumulation on the
PE; 4-direction merge via zero-masked ReduceScatter over the 4 cores of
each batch; post-stage (LN, z-gate, out_proj, reverse-mask, post_proj,
gated residual) on its L-quarter q = c%4.

Perf changes vs the 679us-HW baseline (480us -> 381us on the TimelineSim
cost model, ~-20%):
- Pool/DVE split retuned: Pool takes dtu (except the cold-start section),
  b16 for n>=5 and p16 for n<2; everything else stays on DVE (a fuller
  Pool was measurably latency-coupling the sections)
- delta-prep emission is software-pipelined: each (k,t1) softplus prep and
  the next direction's (k+1,t0) prep are emitted mid-section (after the
  n=4 iteration of the preceding section), so the in-order Activation
  stream runs the section's urgent a16 exps first and the preps fill Act
  slack during the scans
- the in_proj x-half is folded into the depthwise-conv weights host-side
  (W_tap[c,d] = in_proj_w[d,c] * conv_w[d,tap]); feat2 is written directly
  into a zero-padded layout and the conv contracts over it, deleting the
  whole in_proj stage (16 matmuls, 8 copies, 17KB SBUF)
- weights arrive in 2 packed DMAs (one bf16, one f32) instead of 19 SWDGE
  loads
- activation-table thrash removed: competing act_func_sets are neutralized
  (index-preserving) so Exp/Ln/Copy/Square share one set, and activations
  are phase-ordered Gelu -> Silu -> Sigmoid -> Exp/Ln world -> final Gelu
  (6 table loads instead of 24)
- delta is kept in bf16 (dtu hits the DVE 2x mode); per-(k,t-half) delta
  tiles triple-buffered so the next direction's softplus overlaps scans
- k0's prep is interleaved with the depthwise conv so the first scan
  section starts ~15us earlier; conv emitted block-major for that
- dtu, b16 (n>=4) and p16 (n<2) run on the otherwise-idle GPSIMD (Pool)
  engine; the reduce PSUM is split in two banks' halves so y32 updates
  pipeline; the Ds*u init is fused into k0's y-accumulate
- the canonical-half-1 merge masks + ReduceScatter input DMAs issue right
  after (k3,t0), hidden under the last scan section; masks are split
  Act/DVE; the reverse-mask sigmoid is folded into the z-gate in the
  pre-stage
- post-stage normalization runs in bf16 off SBUF row-broadcasts and is
  pipelined per 512-column chunk; out_proj/post_proj feed Gelu via an
  Act copy that shares the Gelu table
"""

import os
import numpy as np
import ml_dtypes

import concourse.bass as bass
import concourse.mybir as mybir
import concourse.tile as tile
from concourse import bacc
from concourse.bass_utils import run_bass_kernel_spmd
from concourse.bass_interp import get_hw_module
from concourse.hw_specs import get_activation_tables as _gat

# The act-table placement pass greedily picks the first act_func_set that
# contains each activation function; Exp and Ln then land in different
# sets and every Exp<->Ln transition costs a 1.3us table load. Neutralize
# the competing sets (preserving list indices, which walrus interprets as
# act_info.json positions) so Exp/Ln/Copy/Square all resolve to the one
# combined set and the scan phase runs with zero table switches.
_KEEP_TABLES = {"natural_log_exp_and_others", "gelu_and_others",
                "silu_and_others", "sigmoid_and_others"}


def _gat_filtered(arch):
    return {name: (s if name in _KEEP_TABLES else set())
            for name, s in _gat(arch).items()}


bacc.get_activation_tables = _gat_filtered

F32 = mybir.dt.float32
BF16 = mybir.dt.bfloat16
AF = mybir.ActivationFunctionType
OP = mybir.AluOpType

B, C, H, W = 2, 128, 64, 64
L = H * W                      # 4096
DI, N, R, K = 256, 16, 8, 4
TH = L // 2
NQ = L // 4                    # l-quarter for post stage
EPS = 1e-5

bf = lambda x: np.ascontiguousarray(x).astype(ml_dtypes.bfloat16)
f32 = lambda x: np.ascontiguousarray(x, dtype=np.float32)

# packed bf16 weight column offsets
PBF_SEGS = dict(wpre=(0, 128), ipw=(128, 512), convd=(640, 2304),
                xw=(2944, 192), dtw=(3136, 512), opw=(3648, 256),
                wpost=(3904, 128), ident=(4032, 128))
PBF_COLS = 4160
# packed f32 column offsets
PF_SEGS = dict(bpre=(0, 1), convb=(1, 2), dtb=(3, 4), Ak=(7, 32),
               dshalf=(39, 1), m01=(40, 2), lng=(42, 2), lnb=(44, 2),
               bpost=(46, 1), mscbi=(47, 2), gatev=(49, 1))
PF_COLS = 50


def _ap(t, off, dims):
    base = t[:]
    return bass.AP(tensor=base.tensor, offset=base.offset + off,
                   ap=[base.ap[0]] + [list(d) for d in dims])


# scan-order -> source AP over a canonical [*, L] tile, chunk of `cnt` cols
# starting at scan-col j0 (H-row aligned for k1/k3).
def _xs_src(u_t, k, j0, cnt):
    if k == 0:
        return _ap(u_t, j0, [[1, cnt]])
    if k == 2:
        return _ap(u_t, L - 1 - j0, [[-1, cnt]])
    nw = cnt // H
    w0 = j0 // H
    if k == 1:   # xs1[w*64+h] = u[h*64+w]
        return _ap(u_t, w0, [[1, nw], [W, H]])
    # k == 3: xs3[w*64+h] = u[4095 - 64h - w]
    return _ap(u_t, L - 1 - w0, [[-1, nw], [-W, H]])


def build_program():
    nc = bacc.Bacc("TRN2", target_bir_lowering=False, debug=False,
                   enable_asserts=False, num_devices=8)

    def inp(name, shape, dt=F32):
        return nc.dram_tensor(name, shape, dt, kind="ExternalInput").ap()

    feature = inp("feature", [C, L], BF16)
    featq = inp("featq", [C, NQ])
    mrow = inp("mrow", [1, NQ])
    pbf = inp("pbf", [C, PBF_COLS], BF16)
    pf32 = inp("pf32", [C, PF_COLS])

    out_d = nc.dram_tensor("out", [C, NQ], F32, kind="ExternalOutput").ap()

    with tile.TileContext(nc) as tc:
        with tc.tile_pool(name="cn", bufs=1) as cn, \
             tc.tile_pool(name="wk", bufs=1) as wk, \
             tc.tile_pool(name="sc8", bufs=1) as sc8, \
             tc.tile_pool(name="d2", bufs=2) as d2, \
             tc.tile_pool(name="ps", bufs=2, space="PSUM") as ps, \
             tc.tile_pool(name="dram", bufs=1, space="DRAM") as dram:

            pbf_t = cn.tile([C, PBF_COLS], BF16, tag="pbf", name="pbf")
            nc.sync.dma_start(pbf_t[:], pbf)
            pf_t = cn.tile([C, PF_COLS], F32, tag="pf32", name="pf32")
            nc.sync.dma_start(pf_t[:], pf32)

            def wbf(key):
                o, w = PBF_SEGS[key]
                return pbf_t[:, o:o + w]

            def wf(key, j0, j1):
                o, _ = PF_SEGS[key]
                return pf_t[:, o + j0:o + j1]

            wpre_t = wbf("wpre")
            ipw_t = wbf("ipw")
            convd_t = wbf("convd")
            xw_t = wbf("xw")
            dtw_t = pbf_t[0:R, PBF_SEGS["dtw"][0]:PBF_SEGS["dtw"][0] + 512]
            opw_t = wbf("opw")
            wpost_t = wbf("wpost")
            id_t = wbf("ident")

            ones_t = cn.tile([128, 1], BF16, tag="ones", name="ones")
            nc.vector.memset(ones_t[:], 1.0)
            onesr_t = cn.tile([1, 128], BF16, tag="onesr", name="onesr")
            nc.vector.memset(onesr_t[:], 1.0)
            eps_t = cn.tile([128, 1], F32, tag="epsc", name="epsc")
            nc.vector.memset(eps_t[:], EPS)

            # =========== PRE-STAGE (full DI, this core's batch) ===========
            feat16 = d2.tile([C, L], BF16, tag="xdbl", name="xdbl")
            for c4 in range(0, L, 1024):
                nc.sync.dma_start(feat16[:, c4:c4 + 1024],
                                  feature[:, c4:c4 + 1024])
            featq32 = d2.tile([128, NQ], F32, tag="fq32", name="fq32", bufs=1)
            nc.sync.dma_start(featq32[:], featq)
            mq = d2.tile([128, NQ], F32, tag="dtu", name="dtu")
            nc.sync.dma_start(mq[:], bass.AP(
                tensor=mrow.tensor, offset=mrow.offset, ap=[[0, 128], [1, NQ]]))

            # --- Gelu phase: feat2 written zero-padded; the in_proj x-half
            # is folded into the depthwise-conv weights host-side, so the
            # conv contracts over feat2 directly (one fewer pre stage)
            HP, WP2 = H + 2, W + 2
            feat2p = sc8.tile([128, HP * WP2], BF16, tag="sc8", name="sc8")
            nc.gpsimd.memset(feat2p[:], 0.0)

            def feat2_blk(c2):
                pb = ps.tile([128, 1024], F32, tag="big2", name="big2")
                for c5 in range(0, 1024, 512):
                    nc.tensor.matmul(pb[:, c5:c5 + 512], wpre_t,
                                     feat16[:, c2 + c5:c2 + c5 + 512],
                                     start=True, stop=True)
                h0 = c2 // W
                nc.scalar.activation(
                    _ap(feat2p, (h0 + 1) * WP2 + 1, [[WP2, 16], [1, W]]),
                    pb[:], AF.Gelu, bias=wf("bpre", 0, 1), scale=1.0)

            u16 = [wk.tile([128, L], BF16, tag=f"u{dh}", name=f"u{dh}")
                   for dh in range(2)]

            def conv_blk(blk):
                for dh in range(2):
                    pb = ps.tile([128, 1024], F32, tag="big2", name="big2")
                    for tap in range(9):
                        dy, dx = tap // 3, tap % 3
                        for sub in range(0, 1024, 512):
                            h0 = (blk + sub) // W
                            srcap = _ap(feat2p, (h0 + dy) * WP2 + dx,
                                        [[WP2, 8], [1, W]])
                            nc.tensor.matmul(
                                pb[:, sub:sub + 512],
                                convd_t[:, (dh * 9 + tap) * 128:
                                        (dh * 9 + tap + 1) * 128],
                                srcap, start=(tap == 0), stop=(tap == 8))
                    nc.scalar.activation(u16[dh][:, blk:blk + 1024], pb[:],
                                         AF.Silu, bias=wf("convb", dh, dh + 1),
                                         scale=1.0)

            uown = wk.tile([128, L], BF16, tag="uown", name="uown")
            y32 = wk.tile([128, L], F32, tag="y32", name="y32")

            def own_half(th):
                sl = slice(th * TH, (th + 1) * TH)
                tmpu = d2.tile([128, TH], BF16, tag="p16", name="p16", bufs=3)
                nc.vector.tensor_scalar(uown[:, sl], u16[0][:, sl],
                                        wf("m01", 0, 1), None, OP.mult)
                nc.vector.tensor_scalar(tmpu[:], u16[1][:, sl],
                                        wf("m01", 1, 2), None, OP.mult)
                nc.vector.tensor_tensor(uown[:, sl], uown[:, sl], tmpu[:],
                                        OP.add)

            # =========== SCAN SECTIONS (k = 0..3), Exp/Ln table only =======
            bc_d = [dram.tile([16, L], BF16, tag=f"bc{k}", name=f"bc{k}")
                    for k in range(K)]
            rs_in = dram.tile([8, 128, NQ], BF16, tag="rsin", name="rsin")
            rs_out = dram.tile([2, 128, NQ], BF16, tag="rsout", name="rsout")

            def emit_merge_masks(t):
                for qq in range(2):
                    for j in range(2):
                        q = t * 2 + qq
                        c0 = t * TH + qq * NQ
                        ym = d2.tile([128, NQ], BF16, tag="p16", name="p16",
                                     bufs=3)
                        if j == 0:
                            nc.scalar.activation(ym[:], y32[:, c0:c0 + NQ],
                                                 AF.Copy, bias=0.0,
                                                 scale=wf("m01", j, j + 1))
                        else:
                            nc.vector.tensor_scalar(ym[:], y32[:, c0:c0 + NQ],
                                                    wf("m01", j, j + 1), None,
                                                    OP.mult)
                        nc.sync.dma_start(rs_in[2 * q + j], ym[:])

            xdbl_k = [None] * K
            dl16_k = [[None, None] for _ in range(K)]

            def prep_half(k, th, dve_copies=False):
                # xdbl blocks of this half + B/C rows to DRAM + softplus delta
                if xdbl_k[k] is None:
                    xdbl_k[k] = d2.tile([24, L], BF16, tag="xdbl", name="xdbl")
                xdbl = xdbl_k[k]
                for blk in range(th * TH, (th + 1) * TH, 1024):
                    pb = ps.tile([24, 1024], F32, tag="big2", name="big2")
                    for ci in range(0, 1024, 512):
                        for dh in range(2):
                            nc.tensor.matmul(
                                pb[:, ci:ci + 512],
                                xw_t[:, (dh * K + k) * 24:(dh * K + k + 1) * 24],
                                _xs_src(u16[dh], k, blk + ci, 512),
                                start=(dh == 0), stop=(dh == 1))
                    if dve_copies:
                        nc.vector.tensor_copy(xdbl[:, blk:blk + 1024], pb[:])
                    else:
                        nc.scalar.copy(xdbl[:, blk:blk + 1024], pb[:])
                bcb = bc_d[k][:]
                sl = slice(th * TH, (th + 1) * TH)
                nc.sync.dma_start(
                    bass.AP(tensor=bcb.tensor, offset=bcb.offset + th * TH,
                            ap=[[2 * L, 8], [1, TH]]), xdbl[8:16, sl])
                nc.sync.dma_start(
                    bass.AP(tensor=bcb.tensor, offset=bcb.offset + L + th * TH,
                            ap=[[2 * L, 8], [1, TH]]), xdbl[16:24, sl])
                dl16 = d2.tile([128, TH], BF16, tag="dl16", name="dl16", bufs=4)
                dl16_k[k][th] = dl16
                for c2 in range(0, TH, 1024):
                    pb = ps.tile([128, 1024], F32, tag="big2", name="big2")
                    for c5 in range(0, 1024, 512):
                        nc.tensor.matmul(pb[:, c5:c5 + 512],
                                         dtw_t[:, k * 128:(k + 1) * 128],
                                         xdbl[0:8, th * TH + c2 + c5:
                                              th * TH + c2 + c5 + 512],
                                         start=True, stop=True)
                    nc.scalar.activation(dl16[:, c2:c2 + 1024], pb[:], AF.Exp,
                                         bias=wf("dtb", k, k + 1), scale=1.0)
                nc.scalar.activation(dl16[:], dl16[:], AF.Ln, bias=1.0,
                                     scale=1.0)


            feat2_blk(0)
            feat2_blk(1024)
            feat2_blk(2048)
            feat2_blk(3072)
            conv_blk(0)
            conv_blk(1024)
            own_half(0)
            prep_half(0, 0, dve_copies=True)
            conv_blk(2048)
            conv_blk(3072)
            own_half(1)

            for k in range(K):
                carry = [None] * 8
                for t in range(2):
                    dl16 = dl16_k[k][t]
                    # dtu_k = delta_k * xs_k(own lanes)
                    dtu = d2.tile([128, TH], BF16, tag="dtu", name="dtu")
                    deng = nc.vector if (k == 0 and t == 0) else nc.gpsimd
                    deng.tensor_tensor(dtu[:], dl16[:],
                                       _xs_src(uown, k, t * TH, TH), OP.mult)
                    red = [ps.tile([128, 1024], F32, tag=f"red{i}",
                                   name=f"red{i}", bufs=1) for i in range(2)]
                    for n in range(8):
                        if n == 4:
                            if t == 0:
                                prep_half(k, 1, dve_copies=(k == 0))
                            elif k < K - 1:
                                prep_half(k + 1, 0)
                        if n == 6 and k == 3 and t == 1:
                            emit_merge_masks(1)
                        brt = d2.tile([128, TH], BF16, tag="brt",
                                      name="brt", bufs=4)
                        nc.sync.dma_start(
                            brt[:],
                            bass.AP(tensor=bc_d[k][:].tensor,
                                    offset=bc_d[k][:].offset + n * 2 * L + t * TH,
                                    ap=[[0, 128], [1, TH]]))
                        crt = d2.tile([128, TH], BF16, tag="crt",
                                      name="crt", bufs=4)
                        nc.sync.dma_start(
                            crt[:],
                            bass.AP(tensor=bc_d[k][:].tensor,
                                    offset=bc_d[k][:].offset + n * 2 * L + L + t * TH,
                                    ap=[[0, 128], [1, TH]]))
                        a16 = d2.tile([128, TH], BF16, tag="a16", name="a16", bufs=6)
                        nc.scalar.activation(a16[:], dl16[:],
                                             AF.Exp, bias=0.0,
                                             scale=wf("Ak", k * 8 + n, k * 8 + n + 1))
                        b16 = d2.tile([128, TH], BF16, tag="b16", name="b16", bufs=3)
                        beng = nc.gpsimd if n >= 5 else nc.vector
                        beng.tensor_tensor(b16[:], dtu[:], brt[:], OP.mult)
                        h16 = d2.tile([128, TH], BF16, tag="h16", name="h16",
                                      bufs=3)
                        init = 0.0 if t == 0 else carry[n][:, 0:1]
                        nc.vector.tensor_tensor_scan(h16[:], a16[:], b16[:],
                                                     init, OP.mult, OP.add)
                        if t == 0:
                            cr = d2.tile([128, 1], F32, tag="carry",
                                         name="carry", bufs=10)
                            nc.vector.tensor_copy(cr[:], h16[:, TH - 1:TH])
                            carry[n] = cr
                        p16 = d2.tile([128, TH], BF16, tag="p16", name="p16",
                                      bufs=3)
                        peng = nc.gpsimd if n < 2 else nc.vector
                        peng.tensor_tensor(p16[:], h16[:], crt[:], OP.mult)
                        for c5 in range(0, TH, 512):
                            nc.tensor.matmul(red[c5 // 1024][:, c5 % 1024:
                                                 c5 % 1024 + 512], id_t,
                                             p16[:, c5:c5 + 512],
                                             start=(n == 0), stop=(n == 7))
                    for i in range(2):
                        dst = _xs_src(y32, k, t * TH + i * 1024, 1024)
                        if k == 0:
                            nc.vector.scalar_tensor_tensor(
                                dst, _xs_src(uown, k, t * TH + i * 1024, 1024),
                                wf("dshalf", 0, 1), red[i][:], OP.mult, OP.add)
                        else:
                            nc.vector.tensor_tensor(dst, red[i][:], dst,
                                                    OP.add)

            emit_merge_masks(0)
            nc.gpsimd.collective_compute(
                "ReduceScatter", OP.add,
                replica_groups=[[0, 1, 2, 3], [4, 5, 6, 7]],
                ins=[rs_in.opt()], outs=[rs_out.opt()])

            # z-gate pipeline runs inside the collective window (Act/PE/DVE
            # are idle there); ztail = 0*y32[:,0:1] gates it after the last
            # y accumulate without changing values
            ztail = d2.tile([128, 1], F32, tag="carry", name="carry", bufs=10)
            nc.vector.tensor_scalar(ztail[:], y32[:, 0:1], 0.0, 1.0,
                                    OP.mult, OP.add)
            featq16 = d2.tile([128, NQ], BF16, tag="pe", name="pe", bufs=2)
            nc.scalar.activation(featq16[:], featq32[:], AF.Copy,
                                 bias=0.0, scale=ztail[:])
            fq2 = d2.tile([128, NQ], BF16, tag="pe", name="pe", bufs=2)
            pb = ps.tile([128, 1024], F32, tag="big2", name="big2")
            for c5 in range(0, NQ, 512):
                nc.tensor.matmul(pb[:, c5:c5 + 512], wpre_t,
                                 featq16[:, c5:c5 + 512], start=True, stop=True)
            nc.scalar.activation(fq2[:], pb[:], AF.Gelu,
                                 bias=wf("bpre", 0, 1), scale=1.0)
            zq = []
            for dh in range(2):
                pb = ps.tile([128, 1024], F32, tag="big2", name="big2")
                for c5 in range(0, NQ, 512):
                    nc.tensor.matmul(pb[:, c5:c5 + 512],
                                     ipw_t[:, (2 + dh) * 128:(3 + dh) * 128],
                                     fq2[:, c5:c5 + 512], start=True, stop=True)
                z = d2.tile([128, NQ], BF16, tag="zq", name="zq")
                nc.scalar.activation(z[:], pb[:], AF.Silu)
                zq.append(z)
            m16 = d2.tile([128, NQ], BF16, tag="pe", name="pe", bufs=2)
            nc.scalar.activation(m16[:], mq[:], AF.Sigmoid,
                                 bias=wf("mscbi", 1, 2), scale=wf("mscbi", 0, 1))
            for j in range(2):
                nc.vector.tensor_tensor(zq[j][:], zq[j][:], m16[:], OP.mult)

            ysum = []
            for j in range(2):
                t = d2.tile([128, NQ], BF16, tag="a16", name="a16", bufs=6)
                nc.sync.dma_start(t[:], rs_out[j])
                ysum.append(t)

            # =========== POST-STAGE (this core's l-quarter) ===========
            sq = []
            for j in range(2):
                s = d2.tile([128, NQ], BF16, tag="h16", name="h16", bufs=3)
                nc.scalar.activation(s[:], ysum[j][:], AF.Square)
                sq.append(s)
            mu = d2.tile([1, NQ], F32, tag="dtu", name="dtu")
            e2 = d2.tile([1, NQ], F32, tag="b16", name="b16", bufs=3)
            for which, tiles in ((0, ysum), (1, sq)):
                for c5 in range(0, NQ, 512):
                    pc = ps.tile([1, 512], F32, tag="big2", name="big2")
                    for j in range(2):
                        nc.tensor.matmul(pc[:], ones_t[:],
                                         tiles[j][:, c5:c5 + 512],
                                         start=(j == 0), stop=(j == 1))
                    dst = mu if which == 0 else e2
                    nc.scalar.activation(dst[:, c5:c5 + 512], pc[:], AF.Copy,
                                         bias=0.0, scale=1.0 / 256.0)
            mu2 = d2.tile([1, NQ], F32, tag="b16", name="b16", bufs=3)
            nc.scalar.activation(mu2[:], mu[:], AF.Square)
            var = d2.tile([1, NQ], F32, tag="b16", name="b16", bufs=3)
            nc.vector.tensor_tensor(var[:], e2[:], mu2[:], OP.subtract)
            sd = d2.tile([1, NQ], F32, tag="dtu", name="dtu")
            nc.scalar.activation(sd[:], var[:], AF.Ln, bias=eps_t[0:1, 0:1],
                                 scale=1.0)
            inv = d2.tile([1, NQ], F32, tag="p16", name="p16", bufs=3)
            nc.scalar.activation(inv[:], sd[:], AF.Exp, bias=0.0, scale=-0.5)
            qrow = d2.tile([1, NQ], F32, tag="p16", name="p16", bufs=3)
            nc.vector.tensor_tensor(qrow[:], mu[:], inv[:], OP.mult)

            inv16 = d2.tile([1, NQ], BF16, tag="b16", name="b16", bufs=3)
            nc.scalar.copy(inv16[:], inv[:])
            q16 = d2.tile([1, NQ], BF16, tag="b16", name="b16", bufs=3)
            nc.scalar.copy(q16[:], qrow[:])
            invrep = ps.tile([128, NQ], F32, tag="big2", name="big2")
            qrep = ps.tile([128, NQ], F32, tag="big2", name="big2")
            for c5 in range(0, NQ, 512):
                nc.tensor.matmul(invrep[:, c5:c5 + 512], onesr_t[:],
                                 inv16[0:1, c5:c5 + 512], start=True, stop=True)
                nc.tensor.matmul(qrep[:, c5:c5 + 512], onesr_t[:],
                                 q16[0:1, c5:c5 + 512], start=True, stop=True)
            invsb = d2.tile([128, NQ], BF16, tag="dtu", name="dtu")
            nc.scalar.copy(invsb[:], invrep[:])
            qsb = d2.tile([128, NQ], BF16, tag="dtu", name="dtu")
            nc.scalar.copy(qsb[:], qrep[:])

            # normalize -> z-gate -> out_proj -> mask -> post_proj -> gelu
            # -> gated residual, pipelined per 512-col chunk
            for c5 in range(0, NQ, 512):
                ym16 = []
                for j in range(2):
                    t1 = d2.tile([128, 512], BF16, tag="dl16", name="dl16", bufs=4)
                    nc.vector.tensor_tensor(t1[:], ysum[j][:, c5:c5 + 512],
                                            invsb[:, c5:c5 + 512], OP.mult)
                    nc.vector.tensor_tensor(t1[:], t1[:],
                                            qsb[:, c5:c5 + 512], OP.subtract)
                    yl = d2.tile([128, 512], BF16, tag="brt", name="brt", bufs=4)
                    nc.vector.tensor_scalar(yl[:], t1[:], wf("lng", j, j + 1),
                                            wf("lnb", j, j + 1), OP.mult,
                                            OP.add)
                    ym = d2.tile([128, 512], BF16, tag="h16", name="h16", bufs=3)
                    nc.vector.tensor_tensor(ym[:], yl[:],
                                            zq[j][:, c5:c5 + 512], OP.mult)
                    ym16.append(ym)
                pc = ps.tile([128, 512], F32, tag="red0", name="red0", bufs=1)
                for j in range(2):
                    nc.tensor.matmul(pc[:], opw_t[:, j * 128:(j + 1) * 128],
                                     ym16[j][:], start=(j == 0), stop=(j == 1))
                att = d2.tile([128, 512], BF16, tag="xdbl", name="xdbl")
                nc.scalar.copy(att[:], pc[:])
                pb = ps.tile([128, 1024], F32, tag="big2", name="big2")
                nc.tensor.matmul(pb[:, 0:512], wpost_t, att[:],
                                 start=True, stop=True)
                ref32 = d2.tile([128, 512], F32, tag="rs", name="rs")
                nc.scalar.activation(ref32[:], pb[:, 0:512], AF.Gelu,
                                     bias=wf("bpost", 0, 1), scale=1.0)
                o32 = d2.tile([128, 512], F32, tag="p16", name="p16", bufs=3)
                nc.vector.scalar_tensor_tensor(o32[:], ref32[:],
                                               wf("gatev", 0, 1),
                                               featq32[:, c5:c5 + 512],
                                               OP.mult, OP.add)
                nc.sync.dma_start(
                    bass.AP(tensor=out_d.tensor, offset=out_d.offset + c5,
                            ap=[[NQ, 128], [1, 512]]), o32[:])

    nc.compile()
    nc.m = get_hw_module(nc.m)
    return nc


def make_in_maps(inputs):
    fe = f32(inputs["feature"])
    mask = f32(inputs["mask_pred"])
    s1 = inputs["bn1_gamma"] / np.sqrt(inputs["bn1_var"] + EPS)
    t1 = inputs["bn1_beta"] - inputs["bn1_mean"] * s1
    W1 = inputs["pre_w"] * s1[None, :]
    b1 = inputs["pre_w"] @ t1
    s2 = inputs["pre_g"] / np.sqrt(inputs["pre_v"] + EPS)
    t2 = inputs["pre_b"] - inputs["pre_m"] * s2
    Wpre = W1 * s2[:, None]
    bpre_v = b1 * s2 + t2
    sp = inputs["post_g"] / np.sqrt(inputs["post_v"] + EPS)
    tp = inputs["post_b"] - inputs["post_m"] * sp
    Wpost = inputs["post_w"] * sp[:, None]
    sm = inputs["mbn_g"][0] / np.sqrt(inputs["mbn_v"][0] + EPS)
    tm = inputs["mbn_b"][0] - inputs["mbn_m"][0] * sm
    A = -np.exp(f32(inputs["A_logs"])).reshape(K, DI, N)
    Ds3 = f32(inputs["Ds"]).reshape(K, DI)
    xw_full = f32(inputs["x_proj_w"])
    dtw_full = f32(inputs["dt_proj_w"])
    dtb_full = f32(inputs["dt_proj_b"])
    ipw_full = f32(inputs["in_proj_w"])
    conv_w = f32(inputs["conv_w"])
    opw_full = f32(inputs["out_proj_w"])

    # depthwise conv fused with the in_proj x-half:
    # W_tap,dh[c,d] = in_proj_w[dh*128+d, c] * conv_w[dh*128+d, tap]
    convd = np.zeros((128, 18 * 128), np.float32)
    for dh in range(2):
        ip = ipw_full[dh * 128:(dh + 1) * 128, :]          # [d, c]
        for tap in range(9):
            blk = convd[:, (dh * 9 + tap) * 128:(dh * 9 + tap + 1) * 128]
            blk[:] = ip.T * conv_w[dh * 128:(dh + 1) * 128,
                                   tap // 3, tap % 3][None, :]

    opw = np.zeros((128, 256), np.float32)
    for j in range(2):
        opw[:, j * 128:(j + 1) * 128] = opw_full[:, j * 128:(j + 1) * 128].T
    lng = np.stack([inputs["out_ln_g"][:128], inputs["out_ln_g"][128:]], 1)
    lnb = np.stack([inputs["out_ln_b"][:128], inputs["out_ln_b"][128:]], 1)

    # common packed bf16 weights (per-core dtw/xw filled below)
    pbf_common = np.zeros((128, PBF_COLS), np.float32)

    def setb(key, arr):
        o, w = PBF_SEGS[key]
        pbf_common[:arr.shape[0], o:o + arr.shape[1]] = arr

    setb("wpre", Wpre.T)
    setb("ipw", ipw_full.T)
    setb("convd", convd)
    setb("opw", opw)
    setb("wpost", Wpost.T)
    setb("ident", np.eye(128, dtype=np.float32))

    pf_common = np.zeros((128, PF_COLS), np.float32)

    def setf(key, arr):
        o, w = PF_SEGS[key]
        pf_common[:arr.shape[0], o:o + arr.shape[1]] = arr

    setf("bpre", f32(bpre_v)[:, None])
    setf("convb", np.stack([inputs["conv_b"][:128], inputs["conv_b"][128:]], 1))
    setf("lng", lng)
    setf("lnb", lnb)
    setf("bpost", f32(tp)[:, None])
    setf("mscbi", np.tile(np.array([[-sm, -tm]], np.float32), (128, 1)))
    setf("gatev", np.full((128, 1), inputs["gate"][0], np.float32))

    in_maps = []
    for c in range(8):
        b, dh, nh, q = c // 4, (c % 4) // 2, c % 2, c % 4
        dsl = slice(dh * 128, (dh + 1) * 128)
        sel = np.r_[0:R, R + nh * 8:R + nh * 8 + 8,
                    R + N + nh * 8:R + N + nh * 8 + 8]
        xw_c = np.zeros((128, 2 * K * 24), np.float32)
        for dh2 in range(2):
            for k in range(K):
                xw_c[:, (dh2 * K + k) * 24:(dh2 * K + k + 1) * 24] = \
                    xw_full[k][sel][:, dh2 * 128:(dh2 + 1) * 128].T
        dtw_c = np.zeros((R, K * 128), np.float32)
        for k in range(K):
            dtw_c[:, k * 128:(k + 1) * 128] = dtw_full[k, dsl, :].T
        m01c = np.zeros((128, 2), np.float32)
        m01c[:, dh] = 1.0
        fb = fe[b].reshape(C, L)

        pbf_c = pbf_common.copy()
        o, _ = PBF_SEGS["xw"]
        pbf_c[:, o:o + 2 * K * 24] = xw_c
        o, _ = PBF_SEGS["dtw"]
        pbf_c[:R, o:o + 512] = dtw_c

        pf_c = pf_common.copy()
        o, _ = PF_SEGS["dtb"]
        pf_c[:, o:o + 4] = dtb_full[:, dsl].T
        o, _ = PF_SEGS["Ak"]
        pf_c[:, o:o + 32] = (A[:, dsl, nh * 8:nh * 8 + 8]
                             .transpose(1, 0, 2).reshape(128, K * 8))
        o, _ = PF_SEGS["dshalf"]
        pf_c[:, o:o + 1] = f32(0.5 * Ds3[:, dsl].sum(0))[:, None]
        o, _ = PF_SEGS["m01"]
        pf_c[:, o:o + 2] = m01c

        m = dict(
            feature=bf(fb),
            featq=f32(fb[:, q * NQ:(q + 1) * NQ]),
            mrow=f32(mask[b, 0].reshape(1, L)[:, q * NQ:(q + 1) * NQ]),
            pbf=bf(pbf_c),
            pf32=f32(pf_c),
        )
        in_maps.append(m)
    return in_maps


_CACHE = {}


def kernel(**inputs):
    if "nc" not in _CACHE:
        _CACHE["nc"] = build_program()
    nc = _CACHE["nc"]
    in_maps = make_in_maps(inputs)
    res = run_bass_kernel_spmd(nc, in_maps, list(range(8)))
    out = np.empty((B, C, H, W), np.float32)
    for c in range(8):
        b, q = c // 4, c % 4
        out[b].reshape(C, L)[:, q * NQ:(q + 1) * NQ] = res.results[c]["out"]
    return out



# revision 7
# speedup vs baseline: 1.0008x; 1.0008x over previous
"""Trainium2 Bass kernel for nn_CSRA (SS2D/Mamba-style cross-scan module).

Self-contained: builds an SPMD Bass/Tile program for 8 NeuronCores, shards
inputs host-side, runs via run_bass_kernel_spmd, reassembles the output.

Sharding: core c -> (b = c//4, dh = (c%4)//2, nh = c%2).
Every core: full pre-stage for its batch b (BN+pre_proj+in_proj+depthwise
conv, all-DI); then 4 scan-direction sections over its (d-half, n-half)
lanes; per-chunk n-reduction via identity-matmul PSUM accumulation on the
PE; 4-direction merge via zero-masked ReduceScatter over the 4 cores of
each batch; post-stage (LN, z-gate, out_proj, reverse-mask, post_proj,
gated residual) on its L-quarter q = c%4.

Perf changes vs the 679us-HW baseline (480us -> 381us on the TimelineSim
cost model, ~-20%):
- Pool/DVE split retuned: Pool takes dtu (except the cold-start section),
  b16 for n>=5 and p16 for n<2; everything else stays on DVE (a fuller
  Pool was measurably latency-coupling the sections)
- delta-prep emission is software-pipelined: each (k,t1) softplus prep and
  the next direction's (k+1,t0) prep are emitted mid-section (after the
  n=4 iteration of the preceding section), so the in-order Activation
  stream runs the section's urgent a16 exps first and the preps fill Act
  slack during the scans
- the in_proj x-half is folded into the depthwise-conv weights host-side
  (W_tap[c,d] = in_proj_w[d,c] * conv_w[d,tap]); feat2 is written directly
  into a zero-padded layout and the conv contracts over it, deleting the
  whole in_proj stage (16 matmuls, 8 copies, 17KB SBUF)
- weights arrive in 2 packed DMAs (one bf16, one f32) instead of 19 SWDGE
  loads
- activation-table thrash removed: competing act_func_sets are neutralized
  (index-preserving) so Exp/Ln/Copy/Square share one set, and activations
  are phase-ordered Gelu -> Silu -> Sigmoid -> Exp/Ln world -> final Gelu
  (6 table loads instead of 24)
- delta is kept in bf16 (dtu hits the DVE 2x mode); per-(k,t-half) delta
  tiles triple-buffered so the next direction's softplus overlaps scans
- k0's prep is interleaved with the depthwise conv so the first scan
  section starts ~15us earlier; conv emitted block-major for that
- dtu, b16 (n>=4) and p16 (n<2) run on the otherwise-idle GPSIMD (Pool)
  engine; the reduce PSUM is split in two banks' halves so y32 updates
  pipeline; the Ds*u init is fused into k0's y-accumulate
- the canonical-half-1 merge masks + ReduceScatter input DMAs issue right
  after (k3,t0), hidden under the last scan section; masks are split
  Act/DVE; the reverse-mask sigmoid is folded into the z-gate in the
  pre-stage
- post-stage normalization runs in bf16 off SBUF row-broadcasts and is
  pipelined per 512-column chunk; out_proj/post_proj feed Gelu via an
  Act copy that shares the Gelu table
"""

import os
import numpy as np
import ml_dtypes

import concourse.bass as bass
import concourse.mybir as mybir
import concourse.tile as tile
from concourse import bacc
from concourse.bass_utils import run_bass_kernel_spmd
from concourse.bass_interp import get_hw_module
from concourse.hw_specs import get_activation_tables as _gat

# The act-table placement pass greedily picks the first act_func_set that
# contains each activation function; Exp and Ln then land in different
# sets and every Exp<->Ln transition costs a 1.3us table load. Neutralize
# the competing sets (preserving list indices, which walrus interprets as
# act_info.json positions) so Exp/Ln/Copy/Square all resolve to the one
# combined set and the scan phase runs with zero table switches.
_KEEP_TABLES = {"natural_log_exp_and_others", "gelu_and_others",
                "silu_and_others", "sigmoid_and_others"}


def _gat_filtered(arch):
    return {name: (s if name in _KEEP_TABLES else set())
            for name, s in _gat(arch).items()}


bacc.get_activation_tables = _gat_filtered

F32 = mybir.dt.float32
BF16 = mybir.dt.bfloat16
AF = mybir.ActivationFunctionType
OP = mybir.AluOpType

B, C, H, W = 2, 128, 64, 64
L = H * W                      # 4096
DI, N, R, K = 256, 16, 8, 4
TH = L // 2
NQ = L // 4                    # l-quarter for post stage
EPS = 1e-5

bf = lambda x: np.ascontiguousarray(x).astype(ml_dtypes.bfloat16)
f32 = lambda x: np.ascontiguousarray(x, dtype=np.float32)

# packed bf16 weight column offsets
PBF_SEGS = dict(wpre=(0, 128), ipw=(128, 512), convd=(640, 2304),
                xw=(2944, 192), dtw=(3136, 512), opw=(3648, 256),
                wpost=(3904, 128), ident=(4032, 128))
PBF_COLS = 4160
# packed f32 column offsets
PF_SEGS = dict(bpre=(0, 1), convb=(1, 2), dtb=(3, 4), Ak=(7, 32),
               dshalf=(39, 1), m01=(40, 2), lng=(42, 2), lnb=(44, 2),
               bpost=(46, 1), mscbi=(47, 2), gatev=(49, 1))
PF_COLS = 50


def _ap(t, off, dims):
    base = t[:]
    return bass.AP(tensor=base.tensor, offset=base.offset + off,
                   ap=[base.ap[0]] + [list(d) for d in dims])


# scan-order -> source AP over a canonical [*, L] tile, chunk of `cnt` cols
# starting at scan-col j0 (H-row aligned for k1/k3).
def _xs_src(u_t, k, j0, cnt):
    if k == 0:
        return _ap(u_t, j0, [[1, cnt]])
    if k == 2:
        return _ap(u_t, L - 1 - j0, [[-1, cnt]])
    nw = cnt // H
    w0 = j0 // H
    if k == 1:   # xs1[w*64+h] = u[h*64+w]
        return _ap(u_t, w0, [[1, nw], [W, H]])
    # k == 3: xs3[w*64+h] = u[4095 - 64h - w]
    return _ap(u_t, L - 1 - w0, [[-1, nw], [-W, H]])


def build_program():
    nc = bacc.Bacc("TRN2", target_bir_lowering=False, debug=False,
                   enable_asserts=False, num_devices=8)

    def inp(name, shape, dt=F32):
        return nc.dram_tensor(name, shape, dt, kind="ExternalInput").ap()

    feature = inp("feature", [C, L], BF16)
    featq = inp("featq", [C, NQ])
    mrow = inp("mrow", [1, NQ])
    pbf = inp("pbf", [C, PBF_COLS], BF16)
    pf32 = inp("pf32", [C, PF_COLS])

    out_d = nc.dram_tensor("out", [C, NQ], F32, kind="ExternalOutput").ap()

    with tile.TileContext(nc) as tc:
        with tc.tile_pool(name="cn", bufs=1) as cn, \
             tc.tile_pool(name="wk", bufs=1) as wk, \
             tc.tile_pool(name="sc8", bufs=1) as sc8, \
             tc.tile_pool(name="d2", bufs=2) as d2, \
             tc.tile_pool(name="ps", bufs=2, space="PSUM") as ps, \
             tc.tile_pool(name="dram", bufs=1, space="DRAM") as dram:

            pbf_t = cn.tile([C, PBF_COLS], BF16, tag="pbf", name="pbf")
            nc.sync.dma_start(pbf_t[:], pbf)
            pf_t = cn.tile([C, PF_COLS], F32, tag="pf32", name="pf32")
            nc.sync.dma_start(pf_t[:], pf32)

            def wbf(key):
                o, w = PBF_SEGS[key]
                return pbf_t[:, o:o + w]

            def wf(key, j0, j1):
                o, _ = PF_SEGS[key]
                return pf_t[:, o + j0:o + j1]

            wpre_t = wbf("wpre")
            ipw_t = wbf("ipw")
            convd_t = wbf("convd")
            xw_t = wbf("xw")
            dtw_t = pbf_t[0:R, PBF_SEGS["dtw"][0]:PBF_SEGS["dtw"][0] + 512]
            opw_t = wbf("opw")
            wpost_t = wbf("wpost")
            id_t = wbf("ident")

            ones_t = cn.tile([128, 1], BF16, tag="ones", name="ones")
            nc.vector.memset(ones_t[:], 1.0)
            onesr_t = cn.tile([1, 128], BF16, tag="onesr", name="onesr")
            nc.vector.memset(onesr_t[:], 1.0)
            eps_t = cn.tile([128, 1], F32, tag="epsc", name="epsc")
            nc.vector.memset(eps_t[:], EPS)

            # =========== PRE-STAGE (full DI, this core's batch) ===========
            feat16 = d2.tile([C, L], BF16, tag="xdbl", name="xdbl")
            for c4 in range(0, L, 1024):
                nc.sync.dma_start(feat16[:, c4:c4 + 1024],
                                  feature[:, c4:c4 + 1024])
            featq32 = d2.tile([128, NQ], F32, tag="fq32", name="fq32", bufs=1)
            nc.sync.dma_start(featq32[:], featq)
            mq = d2.tile([128, NQ], F32, tag="dtu", name="dtu")
            nc.sync.dma_start(mq[:], bass.AP(
                tensor=mrow.tensor, offset=mrow.offset, ap=[[0, 128], [1, NQ]]))

            # --- Gelu phase: feat2 written zero-padded; the in_proj x-half
            # is folded into the depthwise-conv weights host-side, so the
            # conv contracts over feat2 directly (one fewer pre stage)
            HP, WP2 = H + 2, W + 2
            feat2p = sc8.tile([128, HP * WP2], BF16, tag="sc8", name="sc8")
            nc.gpsimd.memset(feat2p[:], 0.0)

            def feat2_blk(c2):
                pb = ps.tile([128, 1024], F32, tag="big2", name="big2")
                for c5 in range(0, 1024, 512):
                    nc.tensor.matmul(pb[:, c5:c5 + 512], wpre_t,
                                     feat16[:, c2 + c5:c2 + c5 + 512],
                                     start=True, stop=True)
                h0 = c2 // W
                nc.scalar.activation(
                    _ap(feat2p, (h0 + 1) * WP2 + 1, [[WP2, 16], [1, W]]),
                    pb[:], AF.Gelu, bias=wf("bpre", 0, 1), scale=1.0)

            u16 = [wk.tile([128, L], BF16, tag=f"u{dh}", name=f"u{dh}")
                   for dh in range(2)]

            def conv_blk(blk):
                for dh in range(2):
                    pb = ps.tile([128, 1024], F32, tag="big2", name="big2")
                    for tap in range(9):
                        dy, dx = tap // 3, tap % 3
                        for sub in range(0, 1024, 512):
                            h0 = (blk + sub) // W
                            srcap = _ap(feat2p, (h0 + dy) * WP2 + dx,
                                        [[WP2, 8], [1, W]])
                            nc.tensor.matmul(
                                pb[:, sub:sub + 512],
                                convd_t[:, (dh * 9 + tap) * 128:
                                        (dh * 9 + tap + 1) * 128],
                                srcap, start=(tap == 0), stop=(tap == 8))
                    nc.scalar.activation(u16[dh][:, blk:blk + 1024], pb[:],
                                         AF.Silu, bias=wf("convb", dh, dh + 1),
                                         scale=1.0)

            uown = wk.tile([128, L], BF16, tag="uown", name="uown")
            y32 = wk.tile([128, L], F32, tag="y32", name="y32")

            def own_half(th):
                sl = slice(th * TH, (th + 1) * TH)
                tmpu = d2.tile([128, TH], BF16, tag="p16", name="p16", bufs=3)
                nc.vector.tensor_scalar(uown[:, sl], u16[0][:, sl],
                                        wf("m01", 0, 1), None, OP.mult)
                nc.vector.tensor_scalar(tmpu[:], u16[1][:, sl],
                                        wf("m01", 1, 2), None, OP.mult)
                nc.vector.tensor_tensor(uown[:, sl], uown[:, sl], tmpu[:],
                                        OP.add)

            # =========== SCAN SECTIONS (k = 0..3), Exp/Ln table only =======
            bc_d = [dram.tile([16, L], BF16, tag=f"bc{k}", name=f"bc{k}")
                    for k in range(K)]
            rs_in = dram.tile([8, 128, NQ], BF16, tag="rsin", name="rsin")
            rs_out = dram.tile([2, 128, NQ], BF16, tag="rsout", name="rsout")

            def emit_merge_masks(t):
                for qq in range(2):
                    for j in range(2):
                        q = t * 2 + qq
                        c0 = t * TH + qq * NQ
                        ym = d2.tile([128, NQ], BF16, tag="p16", name="p16",
                                     bufs=3)
                        if j == 0:
                            nc.scalar.activation(ym[:], y32[:, c0:c0 + NQ],
                                                 AF.Copy, bias=0.0,
                                                 scale=wf("m01", j, j + 1))
                        else:
                            nc.vector.tensor_scalar(ym[:], y32[:, c0:c0 + NQ],
                                                    wf("m01", j, j + 1), None,
                                                    OP.mult)
                        nc.sync.dma_start(rs_in[2 * q + j], ym[:])

            xdbl_k = [None] * K
            dl16_k = [[None, None] for _ in range(K)]

            def prep_half(k, th, dve_copies=False):
                # xdbl blocks of this half + B/C rows to DRAM + softplus delta
                if xdbl_k[k] is None:
                    xdbl_k[k] = d2.tile([24, L], BF16, tag="xdbl", name="xdbl")
                xdbl = xdbl_k[k]
                for blk in range(th * TH, (th + 1) * TH, 1024):
                    pb = ps.tile([24, 1024], F32, tag="big2", name="big2")
                    for ci in range(0, 1024, 512):
                        for dh in range(2):
                            nc.tensor.matmul(
                                pb[:, ci:ci + 512],
                                xw_t[:, (dh * K + k) * 24:(dh * K + k + 1) * 24],
                                _xs_src(u16[dh], k, blk + ci, 512),
                                start=(dh == 0), stop=(dh == 1))
                    if dve_copies:
                        nc.vector.tensor_copy(xdbl[:, blk:blk + 1024], pb[:])
                    else:
                        nc.scalar.copy(xdbl[:, blk:blk + 1024], pb[:])
                bcb = bc_d[k][:]
                sl = slice(th * TH, (th + 1) * TH)
                nc.sync.dma_start(
                    bass.AP(tensor=bcb.tensor, offset=bcb.offset + th * TH,
                            ap=[[2 * L, 8], [1, TH]]), xdbl[8:16, sl])
                nc.sync.dma_start(
                    bass.AP(tensor=bcb.tensor, offset=bcb.offset + L + th * TH,
                            ap=[[2 * L, 8], [1, TH]]), xdbl[16:24, sl])
                dl16 = d2.tile([128, TH], BF16, tag="dl16", name="dl16", bufs=4)
                dl16_k[k][th] = dl16
                for c2 in range(0, TH, 1024):
                    pb = ps.tile([128, 1024], F32, tag="big2", name="big2")
                    for c5 in range(0, 1024, 512):
                        nc.tensor.matmul(pb[:, c5:c5 + 512],
                                         dtw_t[:, k * 128:(k + 1) * 128],
                                         xdbl[0:8, th * TH + c2 + c5:
                                              th * TH + c2 + c5 + 512],
                                         start=True, stop=True)
                    nc.scalar.activation(dl16[:, c2:c2 + 1024], pb[:], AF.Exp,
                                         bias=wf("dtb", k, k + 1), scale=1.0)
                nc.scalar.activation(dl16[:], dl16[:], AF.Ln, bias=1.0,
                                     scale=1.0)


            feat2_blk(0)
            feat2_blk(1024)
            feat2_blk(2048)
            feat2_blk(3072)
            conv_blk(0)
            conv_blk(1024)
            own_half(0)
            prep_half(0, 0, dve_copies=True)
            conv_blk(2048)
            conv_blk(3072)
            own_half(1)

            for k in range(K):
                carry = [None] * 8
                for t in range(2):
                    dl16 = dl16_k[k][t]
                    # dtu_k = delta_k * xs_k(own lanes)
                    dtu = d2.tile([128, TH], BF16, tag="dtu", name="dtu")
                    deng = nc.vector if (k == 0 and t == 0) else nc.gpsimd
                    deng.tensor_tensor(dtu[:], dl16[:],
                                       _xs_src(uown, k, t * TH, TH), OP.mult)
                    red = [ps.tile([128, 1024], F32, tag=f"red{i}",
                                   name=f"red{i}", bufs=1) for i in range(2)]
                    for n in range(8):
                        if n == 4:
                            if t == 0:
                                prep_half(k, 1, dve_copies=(k == 0))
                            elif k < K - 1:
                                prep_half(k + 1, 0)
                        if n == 6 and k == 3 and t == 1:
                            emit_merge_masks(1)
                        brt = d2.tile([128, TH], BF16, tag="brt",
                                      name="brt", bufs=4)
                        nc.sync.dma_start(
                            brt[:],
                            bass.AP(tensor=bc_d[k][:].tensor,
                                    offset=bc_d[k][:].offset + n * 2 * L + t * TH,
                                    ap=[[0, 128], [1, TH]]))
                        crt = d2.tile([128, TH], BF16, tag="crt",
                                      name="crt", bufs=4)
                        nc.sync.dma_start(
                            crt[:],
                            bass.AP(tensor=bc_d[k][:].tensor,
                                    offset=bc_d[k][:].offset + n * 2 * L + L + t * TH,
                                    ap=[[0, 128], [1, TH]]))
                        a16 = d2.tile([128, TH], BF16, tag="a16", name="a16", bufs=6)
                        nc.scalar.activation(a16[:], dl16[:],
                                             AF.Exp, bias=0.0,
                                             scale=wf("Ak", k * 8 + n, k * 8 + n + 1))
                        b16 = d2.tile([128, TH], BF16, tag="b16", name="b16", bufs=3)
                        beng = nc.gpsimd if n >= 5 else nc.vector
                        beng.tensor_tensor(b16[:], dtu[:], brt[:], OP.mult)
                        h16 = d2.tile([128, TH], BF16, tag="h16", name="h16",
                                      bufs=3)
                        init = 0.0 if t == 0 else carry[n][:, 0:1]
                        nc.vector.tensor_tensor_scan(h16[:], a16[:], b16[:],
                                                     init, OP.mult, OP.add)
                        if t == 0:
                            cr = d2.tile([128, 1], F32, tag="carry",
                                         name="carry", bufs=10)
                            nc.vector.tensor_copy(cr[:], h16[:, TH - 1:TH])
                            carry[n] = cr
                        p16 = d2.tile([128, TH], BF16, tag="p16", name="p16",
                                      bufs=3)
                        peng = nc.gpsimd if n < 2 else nc.vector
                        peng.tensor_tensor(p16[:], h16[:], crt[:], OP.mult)
                        for c5 in range(0, TH, 512):
                            nc.tensor.matmul(red[c5 // 1024][:, c5 % 1024:
                                                 c5 % 1024 + 512], id_t,
                                             p16[:, c5:c5 + 512],
                                             start=(n == 0), stop=(n == 7))
                    for i in range(2):
                        dst = _xs_src(y32, k, t * TH + i * 1024, 1024)
                        if k == 0:
                            nc.vector.scalar_tensor_tensor(
                                dst, _xs_src(uown, k, t * TH + i * 1024, 1024),
                                wf("dshalf", 0, 1), red[i][:], OP.mult, OP.add)
                        else:
                            nc.vector.tensor_tensor(dst, red[i][:], dst,
                                                    OP.add)

            emit_merge_masks(0)
            nc.gpsimd.collective_compute(
                "ReduceScatter", OP.add,
                replica_groups=[[0, 1, 2, 3], [4, 5, 6, 7]],
                ins=[rs_in.opt()], outs=[rs_out.opt()])

            # z-gate pipeline runs inside the collective window (Act/PE/DVE
            # are idle there); ztail = 0*y32[:,0:1] gates it after the last
            # y accumulate without changing values
            ztail = d2.tile([128, 1], F32, tag="carry", name="carry", bufs=10)
            nc.vector.tensor_scalar(ztail[:], y32[:, 0:1], 0.0, 1.0,
                                    OP.mult, OP.add)
            featq16 = d2.tile([128, NQ], BF16, tag="pe", name="pe", bufs=2)
            nc.scalar.activation(featq16[:], featq32[:], AF.Copy,
                                 bias=0.0, scale=ztail[:])
            fq2 = d2.tile([128, NQ], BF16, tag="pe", name="pe", bufs=2)
            pb = ps.tile([128, 1024], F32, tag="big2", name="big2")
            for c5 in range(0, NQ, 512):
                nc.tensor.matmul(pb[:, c5:c5 + 512], wpre_t,
                                 featq16[:, c5:c5 + 512], start=True, stop=True)
            nc.scalar.activation(fq2[:], pb[:], AF.Gelu,
                                 bias=wf("bpre", 0, 1), scale=1.0)
            zq = []
            for dh in range(2):
                pb = ps.tile([128, 1024], F32, tag="big2", name="big2")
                for c5 in range(0, NQ, 512):
                    nc.tensor.matmul(pb[:, c5:c5 + 512],
                                     ipw_t[:, (2 + dh) * 128:(3 + dh) * 128],
                                     fq2[:, c5:c5 + 512], start=True, stop=True)
                z = d2.tile([128, NQ], BF16, tag="zq", name="zq")
                nc.scalar.activation(z[:], pb[:], AF.Silu)
                zq.append(z)
            m16 = d2.tile([128, NQ], BF16, tag="pe", name="pe", bufs=2)
            nc.scalar.activation(m16[:], mq[:], AF.Sigmoid,
                                 bias=wf("mscbi", 1, 2), scale=wf("mscbi", 0, 1))
            expre = d2.tile([1, 1], F32, tag="carry", name="carry", bufs=10)
            nc.scalar.activation(expre[:], m16[0:1, 0:1], AF.Square)
            for j in range(2):
                nc.vector.tensor_tensor(zq[j][:], zq[j][:], m16[:], OP.mult)

            ysum = []
            for j in range(2):
                t = d2.tile([128, NQ], BF16, tag="a16", name="a16", bufs=6)
                (nc.sync if j == 0 else nc.scalar).dma_start(t[:], rs_out[j])
                ysum.append(t)

            # =========== POST-STAGE (this core's l-quarter) ===========
            sq = []
            for j in range(2):
                s = d2.tile([128, NQ], BF16, tag="h16", name="h16", bufs=3)
                nc.scalar.activation(s[:], ysum[j][:], AF.Square)
                sq.append(s)
            mu = d2.tile([1, NQ], F32, tag="dtu", name="dtu")
            e2 = d2.tile([1, NQ], F32, tag="b16", name="b16", bufs=3)
            for which, tiles in ((0, ysum), (1, sq)):
                for c5 in range(0, NQ, 512):
                    pc = ps.tile([1, 512], F32, tag="big2", name="big2")
                    for j in range(2):
                        nc.tensor.matmul(pc[:], ones_t[:],
                                         tiles[j][:, c5:c5 + 512],
                                         start=(j == 0), stop=(j == 1))
                    dst = mu if which == 0 else e2
                    nc.scalar.activation(dst[:, c5:c5 + 512], pc[:], AF.Copy,
                                         bias=0.0, scale=1.0 / 256.0)
            mu2 = d2.tile([1, NQ], F32, tag="b16", name="b16", bufs=3)
            nc.scalar.activation(mu2[:], mu[:], AF.Square)
            var = d2.tile([1, NQ], F32, tag="b16", name="b16", bufs=3)
            nc.vector.tensor_tensor(var[:], e2[:], mu2[:], OP.subtract)
            sd = d2.tile([1, NQ], F32, tag="dtu", name="dtu")
            nc.scalar.activation(sd[:], var[:], AF.Ln, bias=eps_t[0:1, 0:1],
                                 scale=1.0)
            inv = d2.tile([1, NQ], F32, tag="p16", name="p16", bufs=3)
            nc.scalar.activation(inv[:], sd[:], AF.Exp, bias=0.0, scale=-0.5)
            qrow = d2.tile([1, NQ], F32, tag="p16", name="p16", bufs=3)
            nc.vector.tensor_tensor(qrow[:], mu[:], inv[:], OP.mult)

            inv16 = d2.tile([1, NQ], BF16, tag="b16", name="b16", bufs=3)
            nc.scalar.copy(inv16[:], inv[:])
            q16 = d2.tile([1, NQ], BF16, tag="b16", name="b16", bufs=3)
            nc.scalar.copy(q16[:], qrow[:])
            invrep = ps.tile([128, NQ], F32, tag="big2", name="big2")
            qrep = ps.tile([128, NQ], F32, tag="big2", name="big2")
            for c5 in range(0, NQ, 512):
                nc.tensor.matmul(invrep[:, c5:c5 + 512], onesr_t[:],
                                 inv16[0:1, c5:c5 + 512], start=True, stop=True)
                nc.tensor.matmul(qrep[:, c5:c5 + 512], onesr_t[:],
                                 q16[0:1, c5:c5 + 512], start=True, stop=True)
            invsb = d2.tile([128, NQ], BF16, tag="dtu", name="dtu")
            nc.scalar.copy(invsb[:], invrep[:])
            qsb = d2.tile([128, NQ], BF16, tag="dtu", name="dtu")
            nc.scalar.copy(qsb[:], qrep[:])

            # normalize -> z-gate -> out_proj -> mask -> post_proj -> gelu
            # -> gated residual, pipelined per 512-col chunk
            for c5 in range(0, NQ, 512):
                ym16 = []
                for j in range(2):
                    t1 = d2.tile([128, 512], BF16, tag="dl16", name="dl16", bufs=4)
                    nc.vector.tensor_tensor(t1[:], ysum[j][:, c5:c5 + 512],
                                            invsb[:, c5:c5 + 512], OP.mult)
                    nc.vector.tensor_tensor(t1[:], t1[:],
                                            qsb[:, c5:c5 + 512], OP.subtract)
                    yl = d2.tile([128, 512], BF16, tag="brt", name="brt", bufs=4)
                    nc.vector.tensor_scalar(yl[:], t1[:], wf("lng", j, j + 1),
                                            wf("lnb", j, j + 1), OP.mult,
                                            OP.add)
                    ym = d2.tile([128, 512], BF16, tag="h16", name="h16", bufs=3)
                    nc.vector.tensor_tensor(ym[:], yl[:],
                                            zq[j][:, c5:c5 + 512], OP.mult)
                    ym16.append(ym)
                pc = ps.tile([128, 512], F32, tag="red0", name="red0", bufs=1)
                for j in range(2):
                    nc.tensor.matmul(pc[:], opw_t[:, j * 128:(j + 1) * 128],
                                     ym16[j][:], start=(j == 0), stop=(j == 1))
                att = d2.tile([128, 512], BF16, tag="xdbl", name="xdbl")
                nc.scalar.copy(att[:], pc[:])
                pb = ps.tile([128, 1024], F32, tag="big2", name="big2")
                nc.tensor.matmul(pb[:, 0:512], wpost_t, att[:],
                                 start=True, stop=True)
                ref32 = d2.tile([128, 512], F32, tag="rs", name="rs")
                nc.scalar.activation(ref32[:], pb[:, 0:512], AF.Gelu,
                                     bias=wf("bpost", 0, 1), scale=1.0)
                o32 = d2.tile([128, 512], F32, tag="p16", name="p16", bufs=3)
                nc.vector.scalar_tensor_tensor(o32[:], ref32[:],
                                               wf("gatev", 0, 1),
                                               featq32[:, c5:c5 + 512],
                                               OP.mult, OP.add)
                nc.sync.dma_start(
                    bass.AP(tensor=out_d.tensor, offset=out_d.offset + c5,
                            ap=[[NQ, 128], [1, 512]]), o32[:])

    nc.compile()
    nc.m = get_hw_module(nc.m)
    return nc


def make_in_maps(inputs):
    fe = f32(inputs["feature"])
    mask = f32(inputs["mask_pred"])
    s1 = inputs["bn1_gamma"] / np.sqrt(inputs["bn1_var"] + EPS)
    t1 = inputs["bn1_beta"] - inputs["bn1_mean"] * s1
    W1 = inputs["pre_w"] * s1[None, :]
    b1 = inputs["pre_w"] @ t1
    s2 = inputs["pre_g"] / np.sqrt(inputs["pre_v"] + EPS)
    t2 = inputs["pre_b"] - inputs["pre_m"] * s2
    Wpre = W1 * s2[:, None]
    bpre_v = b1 * s2 + t2
    sp = inputs["post_g"] / np.sqrt(inputs["post_v"] + EPS)
    tp = inputs["post_b"] - inputs["post_m"] * sp
    Wpost = inputs["post_w"] * sp[:, None]
    sm = inputs["mbn_g"][0] / np.sqrt(inputs["mbn_v"][0] + EPS)
    tm = inputs["mbn_b"][0] - inputs["mbn_m"][0] * sm
    A = -np.exp(f32(inputs["A_logs"])).reshape(K, DI, N)
    Ds3 = f32(inputs["Ds"]).reshape(K, DI)
    xw_full = f32(inputs["x_proj_w"])
    dtw_full = f32(inputs["dt_proj_w"])
    dtb_full = f32(inputs["dt_proj_b"])
    ipw_full = f32(inputs["in_proj_w"])
    conv_w = f32(inputs["conv_w"])
    opw_full = f32(inputs["out_proj_w"])

    # depthwise conv fused with the in_proj x-half:
    # W_tap,dh[c,d] = in_proj_w[dh*128+d, c] * conv_w[dh*128+d, tap]
    convd = np.zeros((128, 18 * 128), np.float32)
    for dh in range(2):
        ip = ipw_full[dh * 128:(dh + 1) * 128, :]          # [d, c]
        for tap in range(9):
            blk = convd[:, (dh * 9 + tap) * 128:(dh * 9 + tap + 1) * 128]
            blk[:] = ip.T * conv_w[dh * 128:(dh + 1) * 128,
                                   tap // 3, tap % 3][None, :]

    opw = np.zeros((128, 256), np.float32)
    for j in range(2):
        opw[:, j * 128:(j + 1) * 128] = opw_full[:, j * 128:(j + 1) * 128].T
    lng = np.stack([inputs["out_ln_g"][:128], inputs["out_ln_g"][128:]], 1)
    lnb = np.stack([inputs["out_ln_b"][:128], inputs["out_ln_b"][128:]], 1)

    # common packed bf16 weights (per-core dtw/xw filled below)
    pbf_common = np.zeros((128, PBF_COLS), np.float32)

    def setb(key, arr):
        o, w = PBF_SEGS[key]
        pbf_common[:arr.shape[0], o:o + arr.shape[1]] = arr

    setb("wpre", Wpre.T)
    setb("ipw", ipw_full.T)
    setb("convd", convd)
    setb("opw", opw)
    setb("wpost", Wpost.T)
    setb("ident", np.eye(128, dtype=np.float32))

    pf_common = np.zeros((128, PF_COLS), np.float32)

    def setf(key, arr):
        o, w = PF_SEGS[key]
        pf_common[:arr.shape[0], o:o + arr.shape[1]] = arr

    setf("bpre", f32(bpre_v)[:, None])
    setf("convb", np.stack([inputs["conv_b"][:128], inputs["conv_b"][128:]], 1))
    setf("lng", lng)
    setf("lnb", lnb)
    setf("bpost", f32(tp)[:, None])
    setf("mscbi", np.tile(np.array([[-sm, -tm]], np.float32), (128, 1)))
    setf("gatev", np.full((128, 1), inputs["gate"][0], np.float32))

    in_maps = []
    for c in range(8):
        b, dh, nh, q = c // 4, (c % 4) // 2, c % 2, c % 4
        dsl = slice(dh * 128, (dh + 1) * 128)
        sel = np.r_[0:R, R + nh * 8:R + nh * 8 + 8,
                    R + N + nh * 8:R + N + nh * 8 + 8]
        xw_c = np.zeros((128, 2 * K * 24), np.float32)
        for dh2 in range(2):
            for k in range(K):
                xw_c[:, (dh2 * K + k) * 24:(dh2 * K + k + 1) * 24] = \
                    xw_full[k][sel][:, dh2 * 128:(dh2 + 1) * 128].T
        dtw_c = np.zeros((R, K * 128), np.float32)
        for k in range(K):
            dtw_c[:, k * 128:(k + 1) * 128] = dtw_full[k, dsl, :].T
        m01c = np.zeros((128, 2), np.float32)
        m01c[:, dh] = 1.0
        fb = fe[b].reshape(C, L)

        pbf_c = pbf_common.copy()
        o, _ = PBF_SEGS["xw"]
        pbf_c[:, o:o + 2 * K * 24] = xw_c
        o, _ = PBF_SEGS["dtw"]
        pbf_c[:R, o:o + 512] = dtw_c

        pf_c = pf_common.copy()
        o, _ = PF_SEGS["dtb"]
        pf_c[:, o:o + 4] = dtb_full[:, dsl].T
        o, _ = PF_SEGS["Ak"]
        pf_c[:, o:o + 32] = (A[:, dsl, nh * 8:nh * 8 + 8]
                             .transpose(1, 0, 2).reshape(128, K * 8))
        o, _ = PF_SEGS["dshalf"]
        pf_c[:, o:o + 1] = f32(0.5 * Ds3[:, dsl].sum(0))[:, None]
        o, _ = PF_SEGS["m01"]
        pf_c[:, o:o + 2] = m01c

        m = dict(
            feature=bf(fb),
            featq=f32(fb[:, q * NQ:(q + 1) * NQ]),
            mrow=f32(mask[b, 0].reshape(1, L)[:, q * NQ:(q + 1) * NQ]),
            pbf=bf(pbf_c),
            pf32=f32(pf_c),
        )
        in_maps.append(m)
    return in_maps


_CACHE = {}


def kernel(**inputs):
    if "nc" not in _CACHE:
        _CACHE["nc"] = build_program()
    nc = _CACHE["nc"]
    in_maps = make_in_maps(inputs)
    res = run_bass_kernel_spmd(nc, in_maps, list(range(8)))
    out = np.empty((B, C, H, W), np.float32)
    for c in range(8):
        b, q = c // 4, c % 4
        out[b].reshape(C, L)[:, q * NQ:(q + 1) * NQ] = res.results[c]["out"]
    return out



# revision 9
# speedup vs baseline: 1.0174x; 1.0166x over previous
"""Trainium2 Bass kernel for nn_CSRA (SS2D/Mamba-style cross-scan module).

Self-contained: builds an SPMD Bass/Tile program for 8 NeuronCores, shards
inputs host-side, runs via run_bass_kernel_spmd, reassembles the output.

Sharding: core c -> (b = c//4, dh = (c%4)//2, nh = c%2).
Every core: full pre-stage for its batch b (BN+pre_proj+in_proj+depthwise
conv, all-DI); then 4 scan-direction sections over its (d-half, n-half)
lanes; per-chunk n-reduction via identity-matmul PSUM accumulation on the
PE; 4-direction merge via zero-masked ReduceScatter over the 4 cores of
each batch; post-stage (LN, z-gate, out_proj, reverse-mask, post_proj,
gated residual) on its L-quarter q = c%4.

Perf changes vs the 679us-HW baseline (480us -> 381us on the TimelineSim
cost model, ~-20%):
- Pool/DVE split retuned: Pool takes dtu (except the cold-start section),
  b16 for n>=5 and p16 for n<2; everything else stays on DVE (a fuller
  Pool was measurably latency-coupling the sections)
- delta-prep emission is software-pipelined: each (k,t1) softplus prep and
  the next direction's (k+1,t0) prep are emitted mid-section (after the
  n=4 iteration of the preceding section), so the in-order Activation
  stream runs the section's urgent a16 exps first and the preps fill Act
  slack during the scans
- the in_proj x-half is folded into the depthwise-conv weights host-side
  (W_tap[c,d] = in_proj_w[d,c] * conv_w[d,tap]); feat2 is written directly
  into a zero-padded layout and the conv contracts over it, deleting the
  whole in_proj stage (16 matmuls, 8 copies, 17KB SBUF)
- weights arrive in 2 packed DMAs (one bf16, one f32) instead of 19 SWDGE
  loads
- activation-table thrash removed: competing act_func_sets are neutralized
  (index-preserving) so Exp/Ln/Copy/Square share one set, and activations
  are phase-ordered Gelu -> Silu -> Sigmoid -> Exp/Ln world -> final Gelu
  (6 table loads instead of 24)
- delta is kept in bf16 (dtu hits the DVE 2x mode); per-(k,t-half) delta
  tiles triple-buffered so the next direction's softplus overlaps scans
- k0's prep is interleaved with the depthwise conv so the first scan
  section starts ~15us earlier; conv emitted block-major for that
- dtu, b16 (n>=4) and p16 (n<2) run on the otherwise-idle GPSIMD (Pool)
  engine; the reduce PSUM is split in two banks' halves so y32 updates
  pipeline; the Ds*u init is fused into k0's y-accumulate
- the canonical-half-1 merge masks + ReduceScatter input DMAs issue right
  after (k3,t0), hidden under the last scan section; masks are split
  Act/DVE; the reverse-mask sigmoid is folded into the z-gate in the
  pre-stage
- post-stage normalization runs in bf16 off SBUF row-broadcasts and is
  pipelined per 512-column chunk; out_proj/post_proj feed Gelu via an
  Act copy that shares the Gelu table
"""

import os
import numpy as np
import ml_dtypes

import concourse.bass as bass
import concourse.mybir as mybir
import concourse.tile as tile
from concourse import bacc
from concourse.bass_utils import run_bass_kernel_spmd
from concourse.bass_interp import get_hw_module
from concourse.hw_specs import get_activation_tables as _gat

# The act-table placement pass greedily picks the first act_func_set that
# contains each activation function; Exp and Ln then land in different
# sets and every Exp<->Ln transition costs a 1.3us table load. Neutralize
# the competing sets (preserving list indices, which walrus interprets as
# act_info.json positions) so Exp/Ln/Copy/Square all resolve to the one
# combined set and the scan phase runs with zero table switches.
_KEEP_TABLES = {"natural_log_exp_and_others", "gelu_and_others",
                "silu_and_others", "sigmoid_and_others"}


def _gat_filtered(arch):
    return {name: (s if name in _KEEP_TABLES else set())
            for name, s in _gat(arch).items()}


bacc.get_activation_tables = _gat_filtered

F32 = mybir.dt.float32
BF16 = mybir.dt.bfloat16
FP8 = mybir.dt.float8e4
AF = mybir.ActivationFunctionType
OP = mybir.AluOpType

B, C, H, W = 2, 128, 64, 64
L = H * W                      # 4096
DI, N, R, K = 256, 16, 8, 4
TH = L // 2
NQ = L // 4                    # l-quarter for post stage
EPS = 1e-5

bf = lambda x: np.ascontiguousarray(x).astype(ml_dtypes.bfloat16)
f32 = lambda x: np.ascontiguousarray(x, dtype=np.float32)

# packed bf16 weight column offsets
PBF_SEGS = dict(wpre=(0, 128), ipw=(128, 512),
                xw=(640, 192), dtw=(832, 512), opw=(1344, 256),
                wpost=(1600, 128), ident=(1728, 128))
PBF_COLS = 1856
# packed f32 column offsets
PF_SEGS = dict(bpre=(0, 1), convb=(1, 2), dtb=(3, 4), Ak=(7, 32),
               dshalf=(39, 1), m01=(40, 2), lng=(42, 2), lnb=(44, 2),
               bpost=(46, 1), mscbi=(47, 2), gatev=(49, 1))
PF_COLS = 50


def _ap(t, off, dims):
    base = t[:]
    return bass.AP(tensor=base.tensor, offset=base.offset + off,
                   ap=[base.ap[0]] + [list(d) for d in dims])


# scan-order -> source AP over a canonical [*, L] tile, chunk of `cnt` cols
# starting at scan-col j0 (H-row aligned for k1/k3).
def _xs_src(u_t, k, j0, cnt):
    if k == 0:
        return _ap(u_t, j0, [[1, cnt]])
    if k == 2:
        return _ap(u_t, L - 1 - j0, [[-1, cnt]])
    nw = cnt // H
    w0 = j0 // H
    if k == 1:   # xs1[w*64+h] = u[h*64+w]
        return _ap(u_t, w0, [[1, nw], [W, H]])
    # k == 3: xs3[w*64+h] = u[4095 - 64h - w]
    return _ap(u_t, L - 1 - w0, [[-1, nw], [-W, H]])


def build_program():
    nc = bacc.Bacc("TRN2", target_bir_lowering=False, debug=False,
                   enable_asserts=False, num_devices=8)

    def inp(name, shape, dt=F32):
        return nc.dram_tensor(name, shape, dt, kind="ExternalInput").ap()

    feature = inp("feature", [C, L], BF16)
    featq = inp("featq", [C, NQ])
    mrow = inp("mrow", [1, NQ])
    pbf = inp("pbf", [C, PBF_COLS], BF16)
    convd8 = inp("convd8", [C, 2304], FP8)
    pf32 = inp("pf32", [C, PF_COLS])

    out_d = nc.dram_tensor("out", [C, NQ], F32, kind="ExternalOutput").ap()

    with tile.TileContext(nc) as tc:
        with tc.tile_pool(name="cn", bufs=1) as cn, \
             tc.tile_pool(name="wk", bufs=1) as wk, \
             tc.tile_pool(name="sc8", bufs=1) as sc8, \
             tc.tile_pool(name="d2", bufs=2) as d2, \
             tc.tile_pool(name="ps", bufs=2, space="PSUM") as ps, \
             tc.tile_pool(name="dram", bufs=1, space="DRAM") as dram:

            pbf_t = cn.tile([C, PBF_COLS], BF16, tag="pbf", name="pbf")
            nc.sync.dma_start(pbf_t[:], pbf)
            pf_t = cn.tile([C, PF_COLS], F32, tag="pf32", name="pf32")
            nc.sync.dma_start(pf_t[:], pf32)
            convd8_t = cn.tile([C, 2304], FP8, tag="convd8", name="convd8")
            nc.sync.dma_start(convd8_t[:], convd8)

            def wbf(key):
                o, w = PBF_SEGS[key]
                return pbf_t[:, o:o + w]

            def wf(key, j0, j1):
                o, _ = PF_SEGS[key]
                return pf_t[:, o + j0:o + j1]

            wpre_t = wbf("wpre")
            ipw_t = wbf("ipw")
            xw_t = wbf("xw")
            dtw_t = pbf_t[0:R, PBF_SEGS["dtw"][0]:PBF_SEGS["dtw"][0] + 512]
            opw_t = wbf("opw")
            wpost_t = wbf("wpost")
            id_t = wbf("ident")

            ones_t = cn.tile([128, 1], BF16, tag="ones", name="ones")
            nc.vector.memset(ones_t[:], 1.0)
            onesr_t = cn.tile([1, 128], BF16, tag="onesr", name="onesr")
            nc.vector.memset(onesr_t[:], 1.0)
            eps_t = cn.tile([128, 1], F32, tag="epsc", name="epsc")
            nc.vector.memset(eps_t[:], EPS)

            # =========== PRE-STAGE (full DI, this core's batch) ===========
            feat16 = d2.tile([C, L], BF16, tag="xdbl", name="xdbl")
            for c4 in range(0, L, 1024):
                nc.sync.dma_start(feat16[:, c4:c4 + 1024],
                                  feature[:, c4:c4 + 1024])
            featq32 = d2.tile([128, NQ], F32, tag="fq32", name="fq32", bufs=1)
            nc.sync.dma_start(featq32[:], featq)
            mq = d2.tile([128, NQ], F32, tag="dtu", name="dtu")
            nc.sync.dma_start(mq[:], bass.AP(
                tensor=mrow.tensor, offset=mrow.offset, ap=[[0, 128], [1, NQ]]))

            # --- Gelu phase: feat2 written zero-padded; the in_proj x-half
            # is folded into the depthwise-conv weights host-side, so the
            # conv contracts over feat2 directly (one fewer pre stage)
            HP, WP2 = H + 2, 80    # row pitch 80: DoubleRow needs pair stride %16==0
            feat2p = sc8.tile([128, HP * WP2], FP8, tag="sc8", name="sc8")
            nc.gpsimd.memset(feat2p[:], 0.0)

            def feat2_blk(c2):
                pb = ps.tile([128, 1024], F32, tag="big2", name="big2")
                for c5 in range(0, 1024, 512):
                    nc.tensor.matmul(pb[:, c5:c5 + 512], wpre_t,
                                     feat16[:, c2 + c5:c2 + c5 + 512],
                                     start=True, stop=True)
                h0 = c2 // W
                nc.scalar.activation(
                    _ap(feat2p, (h0 + 1) * WP2 + 1, [[WP2, 16], [1, W]]),
                    pb[:], AF.Gelu, bias=wf("bpre", 0, 1), scale=1.0)

            u16 = [wk.tile([128, L], BF16, tag=f"u{dh}", name=f"u{dh}")
                   for dh in range(2)]

            def conv_blk(blk):
                # taps 0..7 as 4 DoubleRow pairs (weights for adjacent taps
                # are contiguous in convd; the pair ifmap is one strided AP),
                # tap 8 as a plain matmul -- 10 instructions per (blk,dh)
                # instead of 18, identical bf16 math
                for dh in range(2):
                    pb = ps.tile([128, 1024], F32, tag="big2", name="big2")
                    for sub in range(0, 1024, 512):
                        h0 = (blk + sub) // W
                        # vertical tap pairs (t, t+3): ifmap pair stride WP2
                        # and weight pair stride 384 both satisfy %16==0
                        for t in range(3):
                            srcap = _ap(feat2p, h0 * WP2 + t,
                                        [[WP2, 2], [WP2, 8], [1, W]])
                            wap = _ap(convd8_t, (dh * 9 + t) * 128,
                                      [[384, 2], [1, 128]])
                            nc.tensor.matmul(
                                pb[:, sub:sub + 512], wap,
                                srcap, start=(t == 0), stop=False,
                                perf_mode=mybir.MatmulPerfMode.DoubleRow)
                        for t in range(6, 9):
                            srcap = _ap(feat2p, (h0 + 2) * WP2 + (t - 6),
                                        [[WP2, 8], [1, W]])
                            nc.tensor.matmul(
                                pb[:, sub:sub + 512],
                                convd8_t[:, (dh * 9 + t) * 128:
                                         (dh * 9 + t + 1) * 128],
                                srcap, start=False, stop=(t == 8))
                    nc.scalar.activation(u16[dh][:, blk:blk + 1024], pb[:],
                                         AF.Silu, bias=wf("convb", dh, dh + 1),
                                         scale=1.0 / 256.0)

            uown = wk.tile([128, L], BF16, tag="uown", name="uown")
            y32 = wk.tile([128, L], F32, tag="y32", name="y32")

            def own_half(th):
                sl = slice(th * TH, (th + 1) * TH)
                tmpu = d2.tile([128, TH], BF16, tag="p16", name="p16", bufs=3)
                nc.vector.tensor_scalar(uown[:, sl], u16[0][:, sl],
                                        wf("m01", 0, 1), None, OP.mult)
                nc.vector.tensor_scalar(tmpu[:], u16[1][:, sl],
                                        wf("m01", 1, 2), None, OP.mult)
                nc.vector.tensor_tensor(uown[:, sl], uown[:, sl], tmpu[:],
                                        OP.add)

            # =========== SCAN SECTIONS (k = 0..3), Exp/Ln table only =======
            bc_d = [dram.tile([16, L], BF16, tag=f"bc{k}", name=f"bc{k}")
                    for k in range(K)]
            rs_in = dram.tile([8, 128, NQ], BF16, tag="rsin", name="rsin")
            rs_out = dram.tile([2, 128, NQ], BF16, tag="rsout", name="rsout")

            def emit_merge_masks(t):
                for qq in range(2):
                    for j in range(2):
                        q = t * 2 + qq
                        c0 = t * TH + qq * NQ
                        ym = d2.tile([128, NQ], BF16, tag="p16", name="p16",
                                     bufs=3)
                        if j == 0:
                            nc.scalar.activation(ym[:], y32[:, c0:c0 + NQ],
                                                 AF.Copy, bias=0.0,
                                                 scale=wf("m01", j, j + 1))
                        else:
                            nc.vector.tensor_scalar(ym[:], y32[:, c0:c0 + NQ],
                                                    wf("m01", j, j + 1), None,
                                                    OP.mult)
                        nc.sync.dma_start(rs_in[2 * q + j], ym[:])

            xdbl_k = [None] * K
            dl16_k = [[None, None] for _ in range(K)]

            def prep_half(k, th, dve_copies=False):
                # xdbl blocks of this half + B/C rows to DRAM + softplus delta
                if xdbl_k[k] is None:
                    xdbl_k[k] = d2.tile([24, L], BF16, tag="xdbl", name="xdbl")
                xdbl = xdbl_k[k]
                for blk in range(th * TH, (th + 1) * TH, 1024):
                    pb = ps.tile([24, 1024], F32, tag="big2", name="big2")
                    for ci in range(0, 1024, 512):
                        for dh in range(2):
                            nc.tensor.matmul(
                                pb[:, ci:ci + 512],
                                xw_t[:, (dh * K + k) * 24:(dh * K + k + 1) * 24],
                                _xs_src(u16[dh], k, blk + ci, 512),
                                start=(dh == 0), stop=(dh == 1))
                    if dve_copies:
                        nc.vector.tensor_copy(xdbl[:, blk:blk + 1024], pb[:])
                    else:
                        nc.scalar.copy(xdbl[:, blk:blk + 1024], pb[:])
                bcb = bc_d[k][:]
                sl = slice(th * TH, (th + 1) * TH)
                nc.sync.dma_start(
                    bass.AP(tensor=bcb.tensor, offset=bcb.offset + th * TH,
                            ap=[[2 * L, 8], [1, TH]]), xdbl[8:16, sl])
                nc.sync.dma_start(
                    bass.AP(tensor=bcb.tensor, offset=bcb.offset + L + th * TH,
                            ap=[[2 * L, 8], [1, TH]]), xdbl[16:24, sl])
                dl16 = d2.tile([128, TH], BF16, tag="dl16", name="dl16", bufs=4)
                dl16_k[k][th] = dl16
                for c2 in range(0, TH, 1024):
                    pb = ps.tile([128, 1024], F32, tag="big2", name="big2")
                    for c5 in range(0, 1024, 512):
                        nc.tensor.matmul(pb[:, c5:c5 + 512],
                                         dtw_t[:, k * 128:(k + 1) * 128],
                                         xdbl[0:8, th * TH + c2 + c5:
                                              th * TH + c2 + c5 + 512],
                                         start=True, stop=True)
                    nc.scalar.activation(dl16[:, c2:c2 + 1024], pb[:], AF.Exp,
                                         bias=wf("dtb", k, k + 1), scale=1.0)
                nc.scalar.activation(dl16[:], dl16[:], AF.Ln, bias=1.0,
                                     scale=1.0)


            feat2_blk(0)
            feat2_blk(1024)
            feat2_blk(2048)
            feat2_blk(3072)
            conv_blk(0)
            conv_blk(1024)
            own_half(0)
            prep_half(0, 0, dve_copies=True)
            conv_blk(2048)
            conv_blk(3072)
            own_half(1)

            for k in range(K):
                carry = [None] * 8
                for t in range(2):
                    dl16 = dl16_k[k][t]
                    # dtu_k = delta_k * xs_k(own lanes)
                    dtu = d2.tile([128, TH], BF16, tag="dtu", name="dtu")
                    deng = nc.vector if (k == 0 and t == 0) else nc.gpsimd
                    deng.tensor_tensor(dtu[:], dl16[:],
                                       _xs_src(uown, k, t * TH, TH), OP.mult)
                    red = [ps.tile([128, 1024], F32, tag=f"red{i}",
                                   name=f"red{i}", bufs=1) for i in range(2)]
                    for n in range(8):
                        if n == 4:
                            if t == 0:
                                prep_half(k, 1, dve_copies=(k == 0))
                            elif k < K - 1:
                                prep_half(k + 1, 0)
                        if n == 6 and k == 3 and t == 1:
                            emit_merge_masks(1)
                        brt = d2.tile([128, TH], BF16, tag="brt",
                                      name="brt", bufs=4)
                        nc.sync.dma_start(
                            brt[:],
                            bass.AP(tensor=bc_d[k][:].tensor,
                                    offset=bc_d[k][:].offset + n * 2 * L + t * TH,
                                    ap=[[0, 128], [1, TH]]))
                        crt = d2.tile([128, TH], BF16, tag="crt",
                                      name="crt", bufs=4)
                        nc.sync.dma_start(
                            crt[:],
                            bass.AP(tensor=bc_d[k][:].tensor,
                                    offset=bc_d[k][:].offset + n * 2 * L + L + t * TH,
                                    ap=[[0, 128], [1, TH]]))
                        a16 = d2.tile([128, TH], BF16, tag="a16", name="a16", bufs=6)
                        nc.scalar.activation(a16[:], dl16[:],
                                             AF.Exp, bias=0.0,
                                             scale=wf("Ak", k * 8 + n, k * 8 + n + 1))
                        b16 = d2.tile([128, TH], BF16, tag="b16", name="b16", bufs=3)
                        beng = nc.gpsimd if n >= 5 else nc.vector
                        beng.tensor_tensor(b16[:], dtu[:], brt[:], OP.mult)
                        h16 = d2.tile([128, TH], BF16, tag="h16", name="h16",
                                      bufs=3)
                        init = 0.0 if t == 0 else carry[n][:, 0:1]
                        nc.vector.tensor_tensor_scan(h16[:], a16[:], b16[:],
                                                     init, OP.mult, OP.add)
                        if t == 0:
                            cr = d2.tile([128, 1], F32, tag="carry",
                                         name="carry", bufs=10)
                            nc.vector.tensor_copy(cr[:], h16[:, TH - 1:TH])
                            carry[n] = cr
                        p16 = d2.tile([128, TH], BF16, tag="p16", name="p16",
                                      bufs=3)
                        peng = nc.gpsimd if n < 2 else nc.vector
                        peng.tensor_tensor(p16[:], h16[:], crt[:], OP.mult)
                        for c5 in range(0, TH, 512):
                            nc.tensor.matmul(red[c5 // 1024][:, c5 % 1024:
                                                 c5 % 1024 + 512], id_t,
                                             p16[:, c5:c5 + 512],
                                             start=(n == 0), stop=(n == 7))
                    for i in range(2):
                        dst = _xs_src(y32, k, t * TH + i * 1024, 1024)
                        if k == 0:
                            nc.vector.scalar_tensor_tensor(
                                dst, _xs_src(uown, k, t * TH + i * 1024, 1024),
                                wf("dshalf", 0, 1), red[i][:], OP.mult, OP.add)
                        else:
                            nc.vector.tensor_tensor(dst, red[i][:], dst,
                                                    OP.add)

            emit_merge_masks(0)
            nc.gpsimd.collective_compute(
                "ReduceScatter", OP.add,
                replica_groups=[[0, 1, 2, 3], [4, 5, 6, 7]],
                ins=[rs_in.opt()], outs=[rs_out.opt()])

            # z-gate pipeline runs inside the collective window (Act/PE/DVE
            # are idle there); ztail = 0*y32[:,0:1] gates it after the last
            # y accumulate without changing values
            ztail = d2.tile([128, 1], F32, tag="carry", name="carry", bufs=10)
            nc.vector.tensor_scalar(ztail[:], y32[:, 0:1], 0.0, 1.0,
                                    OP.mult, OP.add)
            featq16 = d2.tile([128, NQ], BF16, tag="pe", name="pe", bufs=2)
            nc.scalar.activation(featq16[:], featq32[:], AF.Copy,
                                 bias=0.0, scale=ztail[:])
            fq2 = d2.tile([128, NQ], BF16, tag="pe", name="pe", bufs=2)
            pb = ps.tile([128, 1024], F32, tag="big2", name="big2")
            for c5 in range(0, NQ, 512):
                nc.tensor.matmul(pb[:, c5:c5 + 512], wpre_t,
                                 featq16[:, c5:c5 + 512], start=True, stop=True)
            nc.scalar.activation(fq2[:], pb[:], AF.Gelu,
                                 bias=wf("bpre", 0, 1), scale=1.0)
            zq = []
            for dh in range(2):
                pb = ps.tile([128, 1024], F32, tag="big2", name="big2")
                for c5 in range(0, NQ, 512):
                    nc.tensor.matmul(pb[:, c5:c5 + 512],
                                     ipw_t[:, (2 + dh) * 128:(3 + dh) * 128],
                                     fq2[:, c5:c5 + 512], start=True, stop=True)
                z = d2.tile([128, NQ], BF16, tag="zq", name="zq")
                nc.scalar.activation(z[:], pb[:], AF.Silu)
                zq.append(z)
            m16 = d2.tile([128, NQ], BF16, tag="pe", name="pe", bufs=2)
            nc.scalar.activation(m16[:], mq[:], AF.Sigmoid,
                                 bias=wf("mscbi", 1, 2), scale=wf("mscbi", 0, 1))
            expre = d2.tile([1, 1], F32, tag="carry", name="carry", bufs=10)
            nc.scalar.activation(expre[:], m16[0:1, 0:1], AF.Square)
            for j in range(2):
                nc.vector.tensor_tensor(zq[j][:], zq[j][:], m16[:], OP.mult)

            ysum = []
            for j in range(2):
                t = d2.tile([128, NQ], BF16, tag="a16", name="a16", bufs=6)
                (nc.sync if j == 0 else nc.scalar).dma_start(t[:], rs_out[j])
                ysum.append(t)

            # =========== POST-STAGE (this core's l-quarter) ===========
            sq = []
            for j in range(2):
                s = d2.tile([128, NQ], BF16, tag="h16", name="h16", bufs=3)
                nc.scalar.activation(s[:], ysum[j][:], AF.Square)
                sq.append(s)
            mu = d2.tile([1, NQ], F32, tag="dtu", name="dtu")
            e2 = d2.tile([1, NQ], F32, tag="b16", name="b16", bufs=3)
            for which, tiles in ((0, ysum), (1, sq)):
                for c5 in range(0, NQ, 512):
                    pc = ps.tile([1, 512], F32, tag="big2", name="big2")
                    for j in range(2):
                        nc.tensor.matmul(pc[:], ones_t[:],
                                         tiles[j][:, c5:c5 + 512],
                                         start=(j == 0), stop=(j == 1))
                    dst = mu if which == 0 else e2
                    nc.scalar.activation(dst[:, c5:c5 + 512], pc[:], AF.Copy,
                                         bias=0.0, scale=1.0 / 256.0)
            mu2 = d2.tile([1, NQ], F32, tag="b16", name="b16", bufs=3)
            nc.scalar.activation(mu2[:], mu[:], AF.Square)
            var = d2.tile([1, NQ], F32, tag="b16", name="b16", bufs=3)
            nc.vector.tensor_tensor(var[:], e2[:], mu2[:], OP.subtract)
            sd = d2.tile([1, NQ], F32, tag="dtu", name="dtu")
            nc.scalar.activation(sd[:], var[:], AF.Ln, bias=eps_t[0:1, 0:1],
                                 scale=1.0)
            inv = d2.tile([1, NQ], F32, tag="p16", name="p16", bufs=3)
            nc.scalar.activation(inv[:], sd[:], AF.Exp, bias=0.0, scale=-0.5)
            qrow = d2.tile([1, NQ], F32, tag="p16", name="p16", bufs=3)
            nc.vector.tensor_tensor(qrow[:], mu[:], inv[:], OP.mult)

            inv16 = d2.tile([1, NQ], BF16, tag="b16", name="b16", bufs=3)
            nc.scalar.copy(inv16[:], inv[:])
            q16 = d2.tile([1, NQ], BF16, tag="b16", name="b16", bufs=3)
            nc.scalar.copy(q16[:], qrow[:])
            invrep = ps.tile([128, NQ], F32, tag="big2", name="big2")
            qrep = ps.tile([128, NQ], F32, tag="big2", name="big2")
            for c5 in range(0, NQ, 512):
                nc.tensor.matmul(invrep[:, c5:c5 + 512], onesr_t[:],
                                 inv16[0:1, c5:c5 + 512], start=True, stop=True)
                nc.tensor.matmul(qrep[:, c5:c5 + 512], onesr_t[:],
                                 q16[0:1, c5:c5 + 512], start=True, stop=True)
            invsb = d2.tile([128, NQ], BF16, tag="dtu", name="dtu")
            nc.scalar.copy(invsb[:], invrep[:])
            qsb = d2.tile([128, NQ], BF16, tag="dtu", name="dtu")
            nc.scalar.copy(qsb[:], qrep[:])

            # normalize -> z-gate -> out_proj -> mask -> post_proj -> gelu
            # -> gated residual, pipelined per 512-col chunk
            for c5 in range(0, NQ, 512):
                ym16 = []
                for j in range(2):
                    t1 = d2.tile([128, 512], BF16, tag="dl16", name="dl16", bufs=4)
                    nc.vector.tensor_tensor(t1[:], ysum[j][:, c5:c5 + 512],
                                            invsb[:, c5:c5 + 512], OP.mult)
                    nc.vector.tensor_tensor(t1[:], t1[:],
                                            qsb[:, c5:c5 + 512], OP.subtract)
                    yl = d2.tile([128, 512], BF16, tag="brt", name="brt", bufs=4)
                    nc.vector.tensor_scalar(yl[:], t1[:], wf("lng", j, j + 1),
                                            wf("lnb", j, j + 1), OP.mult,
                                            OP.add)
                    ym = d2.tile([128, 512], BF16, tag="h16", name="h16", bufs=3)
                    nc.vector.tensor_tensor(ym[:], yl[:],
                                            zq[j][:, c5:c5 + 512], OP.mult)
                    ym16.append(ym)
                pc = ps.tile([128, 512], F32, tag="red0", name="red0", bufs=1)
                for j in range(2):
                    nc.tensor.matmul(pc[:], opw_t[:, j * 128:(j + 1) * 128],
                                     ym16[j][:], start=(j == 0), stop=(j == 1))
                att = d2.tile([128, 512], BF16, tag="xdbl", name="xdbl")
                nc.scalar.copy(att[:], pc[:])
                pb = ps.tile([128, 1024], F32, tag="big2", name="big2")
                nc.tensor.matmul(pb[:, 0:512], wpost_t, att[:],
                                 start=True, stop=True)
                ref32 = d2.tile([128, 512], F32, tag="rs", name="rs")
                nc.scalar.activation(ref32[:], pb[:, 0:512], AF.Gelu,
                                     bias=wf("bpost", 0, 1), scale=1.0)
                o32 = d2.tile([128, 512], F32, tag="p16", name="p16", bufs=3)
                nc.vector.scalar_tensor_tensor(o32[:], ref32[:],
                                               wf("gatev", 0, 1),
                                               featq32[:, c5:c5 + 512],
                                               OP.mult, OP.add)
                nc.sync.dma_start(
                    bass.AP(tensor=out_d.tensor, offset=out_d.offset + c5,
                            ap=[[NQ, 128], [1, 512]]), o32[:])

    nc.compile()
    nc.m = get_hw_module(nc.m)
    return nc


def make_in_maps(inputs):
    fe = f32(inputs["feature"])
    mask = f32(inputs["mask_pred"])
    s1 = inputs["bn1_gamma"] / np.sqrt(inputs["bn1_var"] + EPS)
    t1 = inputs["bn1_beta"] - inputs["bn1_mean"] * s1
    W1 = inputs["pre_w"] * s1[None, :]
    b1 = inputs["pre_w"] @ t1
    s2 = inputs["pre_g"] / np.sqrt(inputs["pre_v"] + EPS)
    t2 = inputs["pre_b"] - inputs["pre_m"] * s2
    Wpre = W1 * s2[:, None]
    bpre_v = b1 * s2 + t2
    sp = inputs["post_g"] / np.sqrt(inputs["post_v"] + EPS)
    tp = inputs["post_b"] - inputs["post_m"] * sp
    Wpost = inputs["post_w"] * sp[:, None]
    sm = inputs["mbn_g"][0] / np.sqrt(inputs["mbn_v"][0] + EPS)
    tm = inputs["mbn_b"][0] - inputs["mbn_m"][0] * sm
    A = -np.exp(f32(inputs["A_logs"])).reshape(K, DI, N)
    Ds3 = f32(inputs["Ds"]).reshape(K, DI)
    xw_full = f32(inputs["x_proj_w"])
    dtw_full = f32(inputs["dt_proj_w"])
    dtb_full = f32(inputs["dt_proj_b"])
    ipw_full = f32(inputs["in_proj_w"])
    conv_w = f32(inputs["conv_w"])
    opw_full = f32(inputs["out_proj_w"])

    # depthwise conv fused with the in_proj x-half:
    # W_tap,dh[c,d] = in_proj_w[dh*128+d, c] * conv_w[dh*128+d, tap]
    convd = np.zeros((128, 18 * 128), np.float32)
    for dh in range(2):
        ip = ipw_full[dh * 128:(dh + 1) * 128, :]          # [d, c]
        for tap in range(9):
            blk = convd[:, (dh * 9 + tap) * 128:(dh * 9 + tap + 1) * 128]
            blk[:] = ip.T * conv_w[dh * 128:(dh + 1) * 128,
                                   tap // 3, tap % 3][None, :]

    opw = np.zeros((128, 256), np.float32)
    for j in range(2):
        opw[:, j * 128:(j + 1) * 128] = opw_full[:, j * 128:(j + 1) * 128].T
    lng = np.stack([inputs["out_ln_g"][:128], inputs["out_ln_g"][128:]], 1)
    lnb = np.stack([inputs["out_ln_b"][:128], inputs["out_ln_b"][128:]], 1)

    # common packed bf16 weights (per-core dtw/xw filled below)
    pbf_common = np.zeros((128, PBF_COLS), np.float32)

    def setb(key, arr):
        o, w = PBF_SEGS[key]
        pbf_common[:arr.shape[0], o:o + arr.shape[1]] = arr

    setb("wpre", Wpre.T)
    setb("ipw", ipw_full.T)
    setb("opw", opw)
    setb("wpost", Wpost.T)
    setb("ident", np.eye(128, dtype=np.float32))

    pf_common = np.zeros((128, PF_COLS), np.float32)

    def setf(key, arr):
        o, w = PF_SEGS[key]
        pf_common[:arr.shape[0], o:o + arr.shape[1]] = arr

    setf("bpre", f32(bpre_v)[:, None])
    setf("convb", np.stack([inputs["conv_b"][:128], inputs["conv_b"][128:]], 1))
    setf("lng", lng)
    setf("lnb", lnb)
    setf("bpost", f32(tp)[:, None])
    setf("mscbi", np.tile(np.array([[-sm, -tm]], np.float32), (128, 1)))
    setf("gatev", np.full((128, 1), inputs["gate"][0], np.float32))

    convd8_arr = np.ascontiguousarray(
        (convd * 256.0).astype(ml_dtypes.float8_e4m3fn))
    in_maps = []
    for c in range(8):
        b, dh, nh, q = c // 4, (c % 4) // 2, c % 2, c % 4
        dsl = slice(dh * 128, (dh + 1) * 128)
        sel = np.r_[0:R, R + nh * 8:R + nh * 8 + 8,
                    R + N + nh * 8:R + N + nh * 8 + 8]
        xw_c = np.zeros((128, 2 * K * 24), np.float32)
        for dh2 in range(2):
            for k in range(K):
                xw_c[:, (dh2 * K + k) * 24:(dh2 * K + k + 1) * 24] = \
                    xw_full[k][sel][:, dh2 * 128:(dh2 + 1) * 128].T
        dtw_c = np.zeros((R, K * 128), np.float32)
        for k in range(K):
            dtw_c[:, k * 128:(k + 1) * 128] = dtw_full[k, dsl, :].T
        m01c = np.zeros((128, 2), np.float32)
        m01c[:, dh] = 1.0
        fb = fe[b].reshape(C, L)

        pbf_c = pbf_common.copy()
        o, _ = PBF_SEGS["xw"]
        pbf_c[:, o:o + 2 * K * 24] = xw_c
        o, _ = PBF_SEGS["dtw"]
        pbf_c[:R, o:o + 512] = dtw_c

        pf_c = pf_common.copy()
        o, _ = PF_SEGS["dtb"]
        pf_c[:, o:o + 4] = dtb_full[:, dsl].T
        o, _ = PF_SEGS["Ak"]
        pf_c[:, o:o + 32] = (A[:, dsl, nh * 8:nh * 8 + 8]
                             .transpose(1, 0, 2).reshape(128, K * 8))
        o, _ = PF_SEGS["dshalf"]
        pf_c[:, o:o + 1] = f32(0.5 * Ds3[:, dsl].sum(0))[:, None]
        o, _ = PF_SEGS["m01"]
        pf_c[:, o:o + 2] = m01c

        m = dict(
            feature=bf(fb),
            convd8=convd8_arr,
            featq=f32(fb[:, q * NQ:(q + 1) * NQ]),
            mrow=f32(mask[b, 0].reshape(1, L)[:, q * NQ:(q + 1) * NQ]),
            pbf=bf(pbf_c),
            pf32=f32(pf_c),
        )
        in_maps.append(m)
    return in_maps


_CACHE = {}


def kernel(**inputs):
    if "nc" not in _CACHE:
        _CACHE["nc"] = build_program()
    nc = _CACHE["nc"]
    in_maps = make_in_maps(inputs)
    res = run_bass_kernel_spmd(nc, in_maps, list(range(8)))
    out = np.empty((B, C, H, W), np.float32)
    for c in range(8):
        b, q = c // 4, c % 4
        out[b].reshape(C, L)[:, q * NQ:(q + 1) * NQ] = res.results[c]["out"]
    return out



# revision 10
# speedup vs baseline: 1.0303x; 1.0127x over previous
"""Trainium2 Bass kernel for nn_CSRA (SS2D/Mamba-style cross-scan module).

Self-contained: builds an SPMD Bass/Tile program for 8 NeuronCores, shards
inputs host-side, runs via run_bass_kernel_spmd, reassembles the output.

Sharding: core c -> (b = c//4, dh = (c%4)//2, nh = c%2).
Every core: full pre-stage for its batch b (BN+pre_proj+in_proj+depthwise
conv, all-DI); then 4 scan-direction sections over its (d-half, n-half)
lanes; per-chunk n-reduction via identity-matmul PSUM accumulation on the
PE; 4-direction merge via zero-masked ReduceScatter over the 4 cores of
each batch; post-stage (LN, z-gate, out_proj, reverse-mask, post_proj,
gated residual) on its L-quarter q = c%4.

Perf changes vs the 679us-HW baseline (480us -> 381us on the TimelineSim
cost model, ~-20%):
- Pool/DVE split retuned: Pool takes dtu (except the cold-start section),
  b16 for n>=5 and p16 for n<2; everything else stays on DVE (a fuller
  Pool was measurably latency-coupling the sections)
- delta-prep emission is software-pipelined: each (k,t1) softplus prep and
  the next direction's (k+1,t0) prep are emitted mid-section (after the
  n=4 iteration of the preceding section), so the in-order Activation
  stream runs the section's urgent a16 exps first and the preps fill Act
  slack during the scans
- the in_proj x-half is folded into the depthwise-conv weights host-side
  (W_tap[c,d] = in_proj_w[d,c] * conv_w[d,tap]); feat2 is written directly
  into a zero-padded layout and the conv contracts over it, deleting the
  whole in_proj stage (16 matmuls, 8 copies, 17KB SBUF)
- weights arrive in 2 packed DMAs (one bf16, one f32) instead of 19 SWDGE
  loads
- activation-table thrash removed: competing act_func_sets are neutralized
  (index-preserving) so Exp/Ln/Copy/Square share one set, and activations
  are phase-ordered Gelu -> Silu -> Sigmoid -> Exp/Ln world -> final Gelu
  (6 table loads instead of 24)
- delta is kept in bf16 (dtu hits the DVE 2x mode); per-(k,t-half) delta
  tiles triple-buffered so the next direction's softplus overlaps scans
- k0's prep is interleaved with the depthwise conv so the first scan
  section starts ~15us earlier; conv emitted block-major for that
- dtu, b16 (n>=4) and p16 (n<2) run on the otherwise-idle GPSIMD (Pool)
  engine; the reduce PSUM is split in two banks' halves so y32 updates
  pipeline; the Ds*u init is fused into k0's y-accumulate
- the canonical-half-1 merge masks + ReduceScatter input DMAs issue right
  after (k3,t0), hidden under the last scan section; masks are split
  Act/DVE; the reverse-mask sigmoid is folded into the z-gate in the
  pre-stage
- post-stage normalization runs in bf16 off SBUF row-broadcasts and is
  pipelined per 512-column chunk; out_proj/post_proj feed Gelu via an
  Act copy that shares the Gelu table
"""

import os
import numpy as np
import ml_dtypes

import concourse.bass as bass
import concourse.mybir as mybir
import concourse.tile as tile
from concourse import bacc
from concourse.bass_utils import run_bass_kernel_spmd
from concourse.bass_interp import get_hw_module
from concourse.hw_specs import get_activation_tables as _gat

# The act-table placement pass greedily picks the first act_func_set that
# contains each activation function; Exp and Ln then land in different
# sets and every Exp<->Ln transition costs a 1.3us table load. Neutralize
# the competing sets (preserving list indices, which walrus interprets as
# act_info.json positions) so Exp/Ln/Copy/Square all resolve to the one
# combined set and the scan phase runs with zero table switches.
_KEEP_TABLES = {"natural_log_exp_and_others", "gelu_and_others",
                "silu_and_others", "sigmoid_and_others"}


def _gat_filtered(arch):
    return {name: (s if name in _KEEP_TABLES else set())
            for name, s in _gat(arch).items()}


bacc.get_activation_tables = _gat_filtered

F32 = mybir.dt.float32
BF16 = mybir.dt.bfloat16
FP8 = mybir.dt.float8e4
AF = mybir.ActivationFunctionType
OP = mybir.AluOpType

B, C, H, W = 2, 128, 64, 64
L = H * W                      # 4096
DI, N, R, K = 256, 16, 8, 4
TH = L // 2
NQ = L // 4                    # l-quarter for post stage
EPS = 1e-5

bf = lambda x: np.ascontiguousarray(x).astype(ml_dtypes.bfloat16)
f32 = lambda x: np.ascontiguousarray(x, dtype=np.float32)

# packed bf16 weight column offsets
PBF_SEGS = dict(wpre=(0, 128), ipw=(128, 512),
                xw=(640, 192), dtw=(832, 512), opw=(1344, 256),
                wpost=(1600, 128), ident=(1728, 128))
PBF_COLS = 1856
# packed f32 column offsets
PF_SEGS = dict(bpre=(0, 1), convb=(1, 2), dtb=(3, 4), Ak=(7, 32),
               dshalf=(39, 1), m01=(40, 2), lng=(42, 2), lnb=(44, 2),
               bpost=(46, 1), mscbi=(47, 2), gatev=(49, 1))
PF_COLS = 50


def _ap(t, off, dims):
    base = t[:]
    return bass.AP(tensor=base.tensor, offset=base.offset + off,
                   ap=[base.ap[0]] + [list(d) for d in dims])


# scan-order -> source AP over a canonical [*, L] tile, chunk of `cnt` cols
# starting at scan-col j0 (H-row aligned for k1/k3).
def _xs_src(u_t, k, j0, cnt):
    if k == 0:
        return _ap(u_t, j0, [[1, cnt]])
    if k == 2:
        return _ap(u_t, L - 1 - j0, [[-1, cnt]])
    nw = cnt // H
    w0 = j0 // H
    if k == 1:   # xs1[w*64+h] = u[h*64+w]
        return _ap(u_t, w0, [[1, nw], [W, H]])
    # k == 3: xs3[w*64+h] = u[4095 - 64h - w]
    return _ap(u_t, L - 1 - w0, [[-1, nw], [-W, H]])


def build_program():
    nc = bacc.Bacc("TRN2", target_bir_lowering=False, debug=False,
                   enable_asserts=False, num_devices=8)

    def inp(name, shape, dt=F32):
        return nc.dram_tensor(name, shape, dt, kind="ExternalInput").ap()

    feature = inp("feature", [C, L], BF16)
    featq = inp("featq", [C, NQ])
    mrow = inp("mrow", [1, NQ])
    pbf = inp("pbf", [C, PBF_COLS], BF16)
    convd8 = inp("convd8", [C, 2304], FP8)
    pf32 = inp("pf32", [C, PF_COLS])

    out_d = nc.dram_tensor("out", [C, NQ], F32, kind="ExternalOutput").ap()

    with tile.TileContext(nc) as tc:
        with tc.tile_pool(name="cn", bufs=1) as cn, \
             tc.tile_pool(name="wk", bufs=1) as wk, \
             tc.tile_pool(name="sc8", bufs=1) as sc8, \
             tc.tile_pool(name="d2", bufs=2) as d2, \
             tc.tile_pool(name="ps", bufs=2, space="PSUM") as ps, \
             tc.tile_pool(name="dram", bufs=1, space="DRAM") as dram:

            pbf_t = cn.tile([C, PBF_COLS], BF16, tag="pbf", name="pbf")
            nc.sync.dma_start(pbf_t[:], pbf)
            pf_t = cn.tile([C, PF_COLS], F32, tag="pf32", name="pf32")
            nc.sync.dma_start(pf_t[:], pf32)
            convd8_t = cn.tile([C, 2304], FP8, tag="convd8", name="convd8")
            nc.sync.dma_start(convd8_t[:], convd8)

            def wbf(key):
                o, w = PBF_SEGS[key]
                return pbf_t[:, o:o + w]

            def wf(key, j0, j1):
                o, _ = PF_SEGS[key]
                return pf_t[:, o + j0:o + j1]

            wpre_t = wbf("wpre")
            ipw_t = wbf("ipw")
            xw_t = wbf("xw")
            dtw_t = pbf_t[0:R, PBF_SEGS["dtw"][0]:PBF_SEGS["dtw"][0] + 512]
            opw_t = wbf("opw")
            wpost_t = wbf("wpost")
            id_t = wbf("ident")

            ones_t = cn.tile([128, 1], BF16, tag="ones", name="ones")
            nc.vector.memset(ones_t[:], 1.0)
            ones2d_t = cn.tile([128, 128], BF16, tag="ones2d", name="ones2d")
            nc.vector.memset(ones2d_t[:], 1.0)
            onesr_t = cn.tile([1, 128], BF16, tag="onesr", name="onesr")
            nc.vector.memset(onesr_t[:], 1.0)
            eps_t = cn.tile([128, 1], F32, tag="epsc", name="epsc")
            nc.vector.memset(eps_t[:], EPS)

            # =========== PRE-STAGE (full DI, this core's batch) ===========
            feat16 = d2.tile([C, L], BF16, tag="xdbl", name="xdbl")
            for c4 in range(0, L, 1024):
                nc.sync.dma_start(feat16[:, c4:c4 + 1024],
                                  feature[:, c4:c4 + 1024])
            featq32 = d2.tile([128, NQ], F32, tag="fq32", name="fq32", bufs=1)
            nc.sync.dma_start(featq32[:], featq)
            mq = d2.tile([128, NQ], F32, tag="dtu", name="dtu")
            nc.sync.dma_start(mq[:], bass.AP(
                tensor=mrow.tensor, offset=mrow.offset, ap=[[0, 128], [1, NQ]]))

            # --- Gelu phase: feat2 written zero-padded; the in_proj x-half
            # is folded into the depthwise-conv weights host-side, so the
            # conv contracts over feat2 directly (one fewer pre stage)
            HP, WP2 = H + 2, 80    # row pitch 80: DoubleRow needs pair stride %16==0
            feat2p = sc8.tile([128, HP * WP2], FP8, tag="sc8", name="sc8")
            nc.gpsimd.memset(feat2p[:], 0.0)

            def feat2_blk(c2):
                pb = ps.tile([128, 1024], F32, tag="big2", name="big2")
                for c5 in range(0, 1024, 512):
                    nc.tensor.matmul(pb[:, c5:c5 + 512], wpre_t,
                                     feat16[:, c2 + c5:c2 + c5 + 512],
                                     start=True, stop=True)
                h0 = c2 // W
                nc.scalar.activation(
                    _ap(feat2p, (h0 + 1) * WP2 + 1, [[WP2, 16], [1, W]]),
                    pb[:], AF.Gelu, bias=wf("bpre", 0, 1), scale=1.0)

            u16 = [wk.tile([128, L], BF16, tag=f"u{dh}", name=f"u{dh}")
                   for dh in range(2)]

            def conv_blk(blk):
                # taps 0..7 as 4 DoubleRow pairs (weights for adjacent taps
                # are contiguous in convd; the pair ifmap is one strided AP),
                # tap 8 as a plain matmul -- 10 instructions per (blk,dh)
                # instead of 18, identical bf16 math
                for dh in range(2):
                    pb = ps.tile([128, 1024], F32, tag="big2", name="big2")
                    for sub in range(0, 1024, 512):
                        h0 = (blk + sub) // W
                        # vertical tap pairs (t, t+3): ifmap pair stride WP2
                        # and weight pair stride 384 both satisfy %16==0
                        for t in range(3):
                            srcap = _ap(feat2p, h0 * WP2 + t,
                                        [[WP2, 2], [WP2, 8], [1, W]])
                            wap = _ap(convd8_t, (dh * 9 + t) * 128,
                                      [[384, 2], [1, 128]])
                            nc.tensor.matmul(
                                pb[:, sub:sub + 512], wap,
                                srcap, start=(t == 0), stop=False,
                                perf_mode=mybir.MatmulPerfMode.DoubleRow)
                        for t in range(6, 9):
                            srcap = _ap(feat2p, (h0 + 2) * WP2 + (t - 6),
                                        [[WP2, 8], [1, W]])
                            nc.tensor.matmul(
                                pb[:, sub:sub + 512],
                                convd8_t[:, (dh * 9 + t) * 128:
                                         (dh * 9 + t + 1) * 128],
                                srcap, start=False, stop=(t == 8))
                    nc.scalar.activation(u16[dh][:, blk:blk + 1024], pb[:],
                                         AF.Silu, bias=wf("convb", dh, dh + 1),
                                         scale=1.0 / 256.0)

            uown = wk.tile([128, L], BF16, tag="uown", name="uown")
            y32 = wk.tile([128, L], F32, tag="y32", name="y32")

            def own_half(th):
                sl = slice(th * TH, (th + 1) * TH)
                tmpu = d2.tile([128, TH], BF16, tag="p16", name="p16", bufs=3)
                nc.vector.tensor_scalar(uown[:, sl], u16[0][:, sl],
                                        wf("m01", 0, 1), None, OP.mult)
                nc.vector.tensor_scalar(tmpu[:], u16[1][:, sl],
                                        wf("m01", 1, 2), None, OP.mult)
                nc.vector.tensor_tensor(uown[:, sl], uown[:, sl], tmpu[:],
                                        OP.add)

            # =========== SCAN SECTIONS (k = 0..3), Exp/Ln table only =======
            bc_d = [dram.tile([16, L], BF16, tag=f"bc{k}", name=f"bc{k}")
                    for k in range(K)]
            rs_in = dram.tile([8, 128, NQ], BF16, tag="rsin", name="rsin")
            rs_out = dram.tile([2, 128, NQ], BF16, tag="rsout", name="rsout")

            def emit_merge_masks(t):
                for qq in range(2):
                    for j in range(2):
                        q = t * 2 + qq
                        c0 = t * TH + qq * NQ
                        ym = d2.tile([128, NQ], BF16, tag="p16", name="p16",
                                     bufs=3)
                        if j == 0:
                            nc.scalar.activation(ym[:], y32[:, c0:c0 + NQ],
                                                 AF.Copy, bias=0.0,
                                                 scale=wf("m01", j, j + 1))
                        else:
                            nc.vector.tensor_scalar(ym[:], y32[:, c0:c0 + NQ],
                                                    wf("m01", j, j + 1), None,
                                                    OP.mult)
                        nc.sync.dma_start(rs_in[2 * q + j], ym[:])

            xdbl_k = [None] * K
            dl16_k = [[None, None] for _ in range(K)]

            def prep_half(k, th, dve_copies=False):
                # xdbl blocks of this half + B/C rows to DRAM + softplus delta
                if xdbl_k[k] is None:
                    xdbl_k[k] = d2.tile([24, L], BF16, tag="xdbl", name="xdbl")
                xdbl = xdbl_k[k]
                for blk in range(th * TH, (th + 1) * TH, 1024):
                    pb = ps.tile([24, 1024], F32, tag="big2", name="big2")
                    for ci in range(0, 1024, 512):
                        for dh in range(2):
                            nc.tensor.matmul(
                                pb[:, ci:ci + 512],
                                xw_t[:, (dh * K + k) * 24:(dh * K + k + 1) * 24],
                                _xs_src(u16[dh], k, blk + ci, 512),
                                start=(dh == 0), stop=(dh == 1))
                    if dve_copies:
                        nc.vector.tensor_copy(xdbl[:, blk:blk + 1024], pb[:])
                    else:
                        nc.scalar.copy(xdbl[:, blk:blk + 1024], pb[:])
                bcb = bc_d[k][:]
                sl = slice(th * TH, (th + 1) * TH)
                nc.sync.dma_start(
                    bass.AP(tensor=bcb.tensor, offset=bcb.offset + th * TH,
                            ap=[[2 * L, 8], [1, TH]]), xdbl[8:16, sl])
                nc.sync.dma_start(
                    bass.AP(tensor=bcb.tensor, offset=bcb.offset + L + th * TH,
                            ap=[[2 * L, 8], [1, TH]]), xdbl[16:24, sl])
                dl16 = d2.tile([128, TH], BF16, tag="dl16", name="dl16", bufs=4)
                dl16_k[k][th] = dl16
                for c2 in range(0, TH, 1024):
                    pb = ps.tile([128, 1024], F32, tag="big2", name="big2")
                    for c5 in range(0, 1024, 512):
                        nc.tensor.matmul(pb[:, c5:c5 + 512],
                                         dtw_t[:, k * 128:(k + 1) * 128],
                                         xdbl[0:8, th * TH + c2 + c5:
                                              th * TH + c2 + c5 + 512],
                                         start=True, stop=True)
                    nc.scalar.activation(dl16[:, c2:c2 + 1024], pb[:], AF.Exp,
                                         bias=wf("dtb", k, k + 1), scale=1.0)
                nc.scalar.activation(dl16[:], dl16[:], AF.Ln, bias=1.0,
                                     scale=1.0)


            feat2_blk(0)
            feat2_blk(1024)
            feat2_blk(2048)
            feat2_blk(3072)
            conv_blk(0)
            conv_blk(1024)
            own_half(0)
            prep_half(0, 0, dve_copies=True)
            conv_blk(2048)
            conv_blk(3072)
            own_half(1)

            for k in range(K):
                carry = [None] * 8
                for t in range(2):
                    dl16 = dl16_k[k][t]
                    # dtu_k = delta_k * xs_k(own lanes)
                    dtu = d2.tile([128, TH], BF16, tag="dtu", name="dtu")
                    deng = nc.vector if (k == 0 and t == 0) else nc.gpsimd
                    deng.tensor_tensor(dtu[:], dl16[:],
                                       _xs_src(uown, k, t * TH, TH), OP.mult)
                    red = [ps.tile([128, 1024], F32, tag=f"red{i}",
                                   name=f"red{i}", bufs=1) for i in range(2)]
                    for n in range(8):
                        if n == 4:
                            if t == 0:
                                prep_half(k, 1, dve_copies=(k == 0))
                            elif k < K - 1:
                                prep_half(k + 1, 0)
                        if n == 6 and k == 3 and t == 1:
                            emit_merge_masks(1)
                        brt = d2.tile([128, TH], BF16, tag="brt",
                                      name="brt", bufs=4)
                        nc.sync.dma_start(
                            brt[:],
                            bass.AP(tensor=bc_d[k][:].tensor,
                                    offset=bc_d[k][:].offset + n * 2 * L + t * TH,
                                    ap=[[0, 128], [1, TH]]))
                        crt = d2.tile([128, TH], BF16, tag="crt",
                                      name="crt", bufs=4)
                        nc.sync.dma_start(
                            crt[:],
                            bass.AP(tensor=bc_d[k][:].tensor,
                                    offset=bc_d[k][:].offset + n * 2 * L + L + t * TH,
                                    ap=[[0, 128], [1, TH]]))
                        a16 = d2.tile([128, TH], BF16, tag="a16", name="a16", bufs=6)
                        nc.scalar.activation(a16[:], dl16[:],
                                             AF.Exp, bias=0.0,
                                             scale=wf("Ak", k * 8 + n, k * 8 + n + 1))
                        b16 = d2.tile([128, TH], BF16, tag="b16", name="b16", bufs=3)
                        beng = nc.gpsimd if n >= 5 else nc.vector
                        beng.tensor_tensor(b16[:], dtu[:], brt[:], OP.mult)
                        h16 = d2.tile([128, TH], BF16, tag="h16", name="h16",
                                      bufs=3)
                        init = 0.0 if t == 0 else carry[n][:, 0:1]
                        nc.vector.tensor_tensor_scan(h16[:], a16[:], b16[:],
                                                     init, OP.mult, OP.add)
                        if t == 0:
                            cr = d2.tile([128, 1], F32, tag="carry",
                                         name="carry", bufs=10)
                            nc.vector.tensor_copy(cr[:], h16[:, TH - 1:TH])
                            carry[n] = cr
                        p16 = d2.tile([128, TH], BF16, tag="p16", name="p16",
                                      bufs=3)
                        peng = nc.gpsimd if n < 2 else nc.vector
                        peng.tensor_tensor(p16[:], h16[:], crt[:], OP.mult)
                        for c5 in range(0, TH, 512):
                            nc.tensor.matmul(red[c5 // 1024][:, c5 % 1024:
                                                 c5 % 1024 + 512], id_t,
                                             p16[:, c5:c5 + 512],
                                             start=(n == 0), stop=(n == 7))
                    for i in range(2):
                        dst = _xs_src(y32, k, t * TH + i * 1024, 1024)
                        if k == 0:
                            nc.vector.scalar_tensor_tensor(
                                dst, _xs_src(uown, k, t * TH + i * 1024, 1024),
                                wf("dshalf", 0, 1), red[i][:], OP.mult, OP.add)
                        else:
                            nc.vector.tensor_tensor(dst, red[i][:], dst,
                                                    OP.add)

            emit_merge_masks(0)
            nc.gpsimd.collective_compute(
                "ReduceScatter", OP.add,
                replica_groups=[[0, 1, 2, 3], [4, 5, 6, 7]],
                ins=[rs_in.opt()], outs=[rs_out.opt()])

            # z-gate pipeline runs inside the collective window (Act/PE/DVE
            # are idle there); ztail = 0*y32[:,0:1] gates it after the last
            # y accumulate without changing values
            ztail = d2.tile([128, 1], F32, tag="carry", name="carry", bufs=10)
            nc.vector.tensor_scalar(ztail[:], y32[:, 0:1], 0.0, 1.0,
                                    OP.mult, OP.add)
            featq16 = d2.tile([128, NQ], BF16, tag="pe", name="pe", bufs=2)
            nc.scalar.activation(featq16[:], featq32[:], AF.Copy,
                                 bias=0.0, scale=ztail[:])
            fq2 = d2.tile([128, NQ], BF16, tag="pe", name="pe", bufs=2)
            pb = ps.tile([128, 1024], F32, tag="big2", name="big2")
            for c5 in range(0, NQ, 512):
                nc.tensor.matmul(pb[:, c5:c5 + 512], wpre_t,
                                 featq16[:, c5:c5 + 512], start=True, stop=True)
            nc.scalar.activation(fq2[:], pb[:], AF.Gelu,
                                 bias=wf("bpre", 0, 1), scale=1.0)
            zq = []
            for dh in range(2):
                pb = ps.tile([128, 1024], F32, tag="big2", name="big2")
                for c5 in range(0, NQ, 512):
                    nc.tensor.matmul(pb[:, c5:c5 + 512],
                                     ipw_t[:, (2 + dh) * 128:(3 + dh) * 128],
                                     fq2[:, c5:c5 + 512], start=True, stop=True)
                z = d2.tile([128, NQ], BF16, tag="zq", name="zq")
                nc.scalar.activation(z[:], pb[:], AF.Silu)
                zq.append(z)
            m16 = d2.tile([128, NQ], BF16, tag="pe", name="pe", bufs=2)
            nc.scalar.activation(m16[:], mq[:], AF.Sigmoid,
                                 bias=wf("mscbi", 1, 2), scale=wf("mscbi", 0, 1))
            expre = d2.tile([1, 1], F32, tag="carry", name="carry", bufs=10)
            nc.scalar.activation(expre[:], m16[0:1, 0:1], AF.Square)
            for j in range(2):
                nc.vector.tensor_tensor(zq[j][:], zq[j][:], m16[:], OP.mult)

            ysum = []
            for j in range(2):
                t = d2.tile([128, NQ], BF16, tag="a16", name="a16", bufs=6)
                (nc.sync if j == 0 else nc.scalar).dma_start(t[:], rs_out[j])
                ysum.append(t)

            # =========== POST-STAGE (this core's l-quarter) ===========
            sq = []
            for j in range(2):
                s = d2.tile([128, NQ], BF16, tag="h16", name="h16", bufs=3)
                nc.scalar.activation(s[:], ysum[j][:], AF.Square)
                sq.append(s)
            # all-ones weights replicate the d-sum across every output
            # partition in one matmul: stats are born broadcast, which
            # deletes the row->bf16->PE-rebroadcast->SBUF copy chain
            murep = ps.tile([128, NQ], F32, tag="big2", name="big2")
            e2rep = ps.tile([128, NQ], F32, tag="big2", name="big2")
            for which, tiles in ((0, ysum), (1, sq)):
                dstp = murep if which == 0 else e2rep
                for c5 in range(0, NQ, 512):
                    for j in range(2):
                        nc.tensor.matmul(dstp[:, c5:c5 + 512], ones2d_t[:],
                                         tiles[j][:, c5:c5 + 512],
                                         start=(j == 0), stop=(j == 1))
            mu2 = d2.tile([128, NQ], BF16, tag="dtu", name="dtu")
            nc.scalar.activation(mu2[:], murep[:], AF.Square, bias=0.0,
                                 scale=1.0 / 256.0)
            var = d2.tile([128, NQ], F32, tag="b16", name="b16", bufs=3)
            nc.vector.scalar_tensor_tensor(var[:], e2rep[:], 1.0 / 256.0,
                                           mu2[:], OP.mult, OP.subtract)
            sd = d2.tile([128, NQ], F32, tag="p16", name="p16", bufs=3)
            nc.scalar.activation(sd[:], var[:], AF.Ln, bias=eps_t[:, 0:1],
                                 scale=1.0)
            invsb = d2.tile([128, NQ], BF16, tag="dtu", name="dtu")
            nc.scalar.activation(invsb[:], sd[:], AF.Exp, bias=0.0, scale=-0.5)
            qsb = d2.tile([128, NQ], BF16, tag="dtu", name="dtu")
            nc.vector.scalar_tensor_tensor(qsb[:], murep[:], 1.0 / 256.0,
                                           invsb[:], OP.mult, OP.mult)

            # normalize -> z-gate -> out_proj -> mask -> post_proj -> gelu
            # -> gated residual, pipelined per 512-col chunk
            for c5 in range(0, NQ, 512):
                ym16 = []
                for j in range(2):
                    t1 = d2.tile([128, 512], BF16, tag="dl16", name="dl16", bufs=4)
                    nc.vector.tensor_tensor(t1[:], ysum[j][:, c5:c5 + 512],
                                            invsb[:, c5:c5 + 512], OP.mult)
                    nc.vector.tensor_tensor(t1[:], t1[:],
                                            qsb[:, c5:c5 + 512], OP.subtract)
                    yl = d2.tile([128, 512], BF16, tag="brt", name="brt", bufs=4)
                    nc.vector.tensor_scalar(yl[:], t1[:], wf("lng", j, j + 1),
                                            wf("lnb", j, j + 1), OP.mult,
                                            OP.add)
                    ym = d2.tile([128, 512], BF16, tag="h16", name="h16", bufs=3)
                    nc.vector.tensor_tensor(ym[:], yl[:],
                                            zq[j][:, c5:c5 + 512], OP.mult)
                    ym16.append(ym)
                pc = ps.tile([128, 512], F32, tag="red0", name="red0", bufs=1)
                for j in range(2):
                    nc.tensor.matmul(pc[:], opw_t[:, j * 128:(j + 1) * 128],
                                     ym16[j][:], start=(j == 0), stop=(j == 1))
                att = d2.tile([128, 512], BF16, tag="xdbl", name="xdbl")
                nc.scalar.copy(att[:], pc[:])
                pb = ps.tile([128, 1024], F32, tag="big2", name="big2")
                nc.tensor.matmul(pb[:, 0:512], wpost_t, att[:],
                                 start=True, stop=True)
                ref32 = d2.tile([128, 512], F32, tag="rs", name="rs")
                nc.scalar.activation(ref32[:], pb[:, 0:512], AF.Gelu,
                                     bias=wf("bpost", 0, 1), scale=1.0)
                o32 = d2.tile([128, 512], F32, tag="p16", name="p16", bufs=3)
                nc.vector.scalar_tensor_tensor(o32[:], ref32[:],
                                               wf("gatev", 0, 1),
                                               featq32[:, c5:c5 + 512],
                                               OP.mult, OP.add)
                nc.sync.dma_start(
                    bass.AP(tensor=out_d.tensor, offset=out_d.offset + c5,
                            ap=[[NQ, 128], [1, 512]]), o32[:])

    nc.compile()
    nc.m = get_hw_module(nc.m)
    return nc


def make_in_maps(inputs):
    fe = f32(inputs["feature"])
    mask = f32(inputs["mask_pred"])
    s1 = inputs["bn1_gamma"] / np.sqrt(inputs["bn1_var"] + EPS)
    t1 = inputs["bn1_beta"] - inputs["bn1_mean"] * s1
    W1 = inputs["pre_w"] * s1[None, :]
    b1 = inputs["pre_w"] @ t1
    s2 = inputs["pre_g"] / np.sqrt(inputs["pre_v"] + EPS)
    t2 = inputs["pre_b"] - inputs["pre_m"] * s2
    Wpre = W1 * s2[:, None]
    bpre_v = b1 * s2 + t2
    sp = inputs["post_g"] / np.sqrt(inputs["post_v"] + EPS)
    tp = inputs["post_b"] - inputs["post_m"] * sp
    Wpost = inputs["post_w"] * sp[:, None]
    sm = inputs["mbn_g"][0] / np.sqrt(inputs["mbn_v"][0] + EPS)
    tm = inputs["mbn_b"][0] - inputs["mbn_m"][0] * sm
    A = -np.exp(f32(inputs["A_logs"])).reshape(K, DI, N)
    Ds3 = f32(inputs["Ds"]).reshape(K, DI)
    xw_full = f32(inputs["x_proj_w"])
    dtw_full = f32(inputs["dt_proj_w"])
    dtb_full = f32(inputs["dt_proj_b"])
    ipw_full = f32(inputs["in_proj_w"])
    conv_w = f32(inputs["conv_w"])
    opw_full = f32(inputs["out_proj_w"])

    # depthwise conv fused with the in_proj x-half:
    # W_tap,dh[c,d] = in_proj_w[dh*128+d, c] * conv_w[dh*128+d, tap]
    convd = np.zeros((128, 18 * 128), np.float32)
    for dh in range(2):
        ip = ipw_full[dh * 128:(dh + 1) * 128, :]          # [d, c]
        for tap in range(9):
            blk = convd[:, (dh * 9 + tap) * 128:(dh * 9 + tap + 1) * 128]
            blk[:] = ip.T * conv_w[dh * 128:(dh + 1) * 128,
                                   tap // 3, tap % 3][None, :]

    opw = np.zeros((128, 256), np.float32)
    for j in range(2):
        opw[:, j * 128:(j + 1) * 128] = opw_full[:, j * 128:(j + 1) * 128].T
    lng = np.stack([inputs["out_ln_g"][:128], inputs["out_ln_g"][128:]], 1)
    lnb = np.stack([inputs["out_ln_b"][:128], inputs["out_ln_b"][128:]], 1)

    # common packed bf16 weights (per-core dtw/xw filled below)
    pbf_common = np.zeros((128, PBF_COLS), np.float32)

    def setb(key, arr):
        o, w = PBF_SEGS[key]
        pbf_common[:arr.shape[0], o:o + arr.shape[1]] = arr

    setb("wpre", Wpre.T)
    setb("ipw", ipw_full.T)
    setb("opw", opw)
    setb("wpost", Wpost.T)
    setb("ident", np.eye(128, dtype=np.float32))

    pf_common = np.zeros((128, PF_COLS), np.float32)

    def setf(key, arr):
        o, w = PF_SEGS[key]
        pf_common[:arr.shape[0], o:o + arr.shape[1]] = arr

    setf("bpre", f32(bpre_v)[:, None])
    setf("convb", np.stack([inputs["conv_b"][:128], inputs["conv_b"][128:]], 1))
    setf("lng", lng)
    setf("lnb", lnb)
    setf("bpost", f32(tp)[:, None])
    setf("mscbi", np.tile(np.array([[-sm, -tm]], np.float32), (128, 1)))
    setf("gatev", np.full((128, 1), inputs["gate"][0], np.float32))

    convd8_arr = np.ascontiguousarray(
        (convd * 256.0).astype(ml_dtypes.float8_e4m3fn))
    in_maps = []
    for c in range(8):
        b, dh, nh, q = c // 4, (c % 4) // 2, c % 2, c % 4
        dsl = slice(dh * 128, (dh + 1) * 128)
        sel = np.r_[0:R, R + nh * 8:R + nh * 8 + 8,
                    R + N + nh * 8:R + N + nh * 8 + 8]
        xw_c = np.zeros((128, 2 * K * 24), np.float32)
        for dh2 in range(2):
            for k in range(K):
                xw_c[:, (dh2 * K + k) * 24:(dh2 * K + k + 1) * 24] = \
                    xw_full[k][sel][:, dh2 * 128:(dh2 + 1) * 128].T
        dtw_c = np.zeros((R, K * 128), np.float32)
        for k in range(K):
            dtw_c[:, k * 128:(k + 1) * 128] = dtw_full[k, dsl, :].T
        m01c = np.zeros((128, 2), np.float32)
        m01c[:, dh] = 1.0
        fb = fe[b].reshape(C, L)

        pbf_c = pbf_common.copy()
        o, _ = PBF_SEGS["xw"]
        pbf_c[:, o:o + 2 * K * 24] = xw_c
        o, _ = PBF_SEGS["dtw"]
        pbf_c[:R, o:o + 512] = dtw_c

        pf_c = pf_common.copy()
        o, _ = PF_SEGS["dtb"]
        pf_c[:, o:o + 4] = dtb_full[:, dsl].T
        o, _ = PF_SEGS["Ak"]
        pf_c[:, o:o + 32] = (A[:, dsl, nh * 8:nh * 8 + 8]
                             .transpose(1, 0, 2).reshape(128, K * 8))
        o, _ = PF_SEGS["dshalf"]
        pf_c[:, o:o + 1] = f32(0.5 * Ds3[:, dsl].sum(0))[:, None]
        o, _ = PF_SEGS["m01"]
        pf_c[:, o:o + 2] = m01c

        m = dict(
            feature=bf(fb),
            convd8=convd8_arr,
            featq=f32(fb[:, q * NQ:(q + 1) * NQ]),
            mrow=f32(mask[b, 0].reshape(1, L)[:, q * NQ:(q + 1) * NQ]),
            pbf=bf(pbf_c),
            pf32=f32(pf_c),
        )
        in_maps.append(m)
    return in_maps


_CACHE = {}


def kernel(**inputs):
    if "nc" not in _CACHE:
        _CACHE["nc"] = build_program()
    nc = _CACHE["nc"]
    in_maps = make_in_maps(inputs)
    res = run_bass_kernel_spmd(nc, in_maps, list(range(8)))
    out = np.empty((B, C, H, W), np.float32)
    for c in range(8):
        b, q = c // 4, c % 4
        out[b].reshape(C, L)[:, q * NQ:(q + 1) * NQ] = res.results[c]["out"]
    return out

